# revision 22
# baseline (speedup 1.0000x reference)
"""ActionMoE Trainium2 kernel.

Contract: kernel(**inputs) takes the FULL unsharded inputs (numpy arrays,
keyed as in setup_inputs()) and returns the full outputs
(final_action, candidate_actions, selection_probs, return_rtg).

Strategy: pure data parallelism over the batch dim (16 batches -> 2 per core,
8 cores, no collectives). Each core runs an identical NEFF over its 4096
tokens. On-chip layout is feature-major ("layout A": features on SBUF/PSUM
partitions, tokens on the free axis), processed in blocks of 512 tokens:

  1. PE-transpose X[tok,1024] -> XT[1024,tok]
  2. Fused first-layer matmul Xt @ Wcat with Wcat=[ew1|sw1|rw1|vw1] (f32r,
     1 cyc/row). Feature tiles are processed in PAIRS sharing one
     [128,1024] psum (2 banks) so one scalar-engine gelu covers both; the
     per-feature bias is accumulated into psum with a K=1 ones-matmul.
  3. Router: exp(logits+rb2) unnormalized; 1/sum via ones-matmul +
     fast-reciprocal; normalization folded into probsA = expR * bcast(1/sum).
  4. Experts: hidden activations pre-scaled by probsA[e] (broadcast onto 128
     partitions via a selector matmul, one DVE mul per expert), then ALL
     expert second-layer matmuls accumulate into a single [32,TB] psum.
  5. Shared expert / residual MLP as small matmuls, biases folded in as K=1
     ones-matmuls or activation bias. Value head: relu on DVE, sigmoid via
     the Exp table (avoids a Sigmoid ACT-table load) + fast reciprocal.
  6. PE-transpose results back to token-major; probs/rtg/mask ride one
     stacked [10,tok] transpose; mask applied token-major. Candidate
     actions are one broadcast DVE multiply per 128 tokens.
"""
import numpy as np

B, S, H, A, E, ED, RH = 16, 2048, 1024, 32, 8, 256, 128
NCORES = 8
NTOK = B * S // NCORES   # tokens per core
TB = 512                 # tokens per block
FT = 20                  # 2560/128 feature tiles: 0-15 experts, 16-17 shared, 18 router, 19 value
SCALING = np.linspace(0.8, 1.2, E, dtype=np.float32)
BF16_G = False  # expert/shared hidden activations + second-layer weights in bf16

_compiled = {}


def build_nc(ntok=NTOK):
    import concourse.tile as tile
    import concourse.mybir as mybir
    from concourse import bacc
    from concourse.masks import make_identity

    f32 = mybir.dt.float32
    f32r = mybir.dt.float32r
    bf16 = mybir.dt.bfloat16
    gdt = bf16 if BF16_G else f32r
    AF = mybir.ActivationFunctionType
    ALU = mybir.AluOpType
    nblk = ntok // TB

    nc = bacc.Bacc("TRN2", target_bir_lowering=False, debug=False,
                   num_devices=NCORES)

    x_d = nc.dram_tensor("x", [ntok, H], f32, kind="ExternalInput")
    ba_d = nc.dram_tensor("ba", [ntok, A], f32, kind="ExternalInput")
    mask_d = nc.dram_tensor("mask", [ntok], f32, kind="ExternalInput")
    ew1_d = nc.dram_tensor("ew1", [E, H, ED], f32, kind="ExternalInput")
    eb1_d = nc.dram_tensor("eb1", [E, ED], f32, kind="ExternalInput")
    ew2_d = nc.dram_tensor("ew2", [E, ED, A], f32, kind="ExternalInput")
    eb2_d = nc.dram_tensor("eb2", [E, A], f32, kind="ExternalInput")
    sw1_d = nc.dram_tensor("sw1", [H, ED], f32, kind="ExternalInput")
    sb1_d = nc.dram_tensor("sb1", [ED], f32, kind="ExternalInput")
    sw2_d = nc.dram_tensor("sw2", [ED, A], f32, kind="ExternalInput")
    sb2_d = nc.dram_tensor("sb2", [A], f32, kind="ExternalInput")
    rw1_d = nc.dram_tensor("rw1", [H, RH], f32, kind="ExternalInput")
    rb1_d = nc.dram_tensor("rb1", [RH], f32, kind="ExternalInput")
    rw2_d = nc.dram_tensor("rw2", [RH, E], f32, kind="ExternalInput")
    rb2_d = nc.dram_tensor("rb2", [E], f32, kind="ExternalInput")
    vw1_d = nc.dram_tensor("vw1", [H, RH], f32, kind="ExternalInput")
    vb1_d = nc.dram_tensor("vb1", [RH], f32, kind="ExternalInput")
    vw2_d = nc.dram_tensor("vw2", [RH, 1], f32, kind="ExternalInput")
    vb2_d = nc.dram_tensor("vb2", [1], f32, kind="ExternalInput")
    mw1_d = nc.dram_tensor("mw1", [A, RH], f32, kind="ExternalInput")
    mb1_d = nc.dram_tensor("mb1", [RH], f32, kind="ExternalInput")
    mw2_d = nc.dram_tensor("mw2", [RH, A], f32, kind="ExternalInput")
    mb2_d = nc.dram_tensor("mb2", [A], f32, kind="ExternalInput")

    fin_d = nc.dram_tensor("fin", [ntok, A], f32, kind="ExternalOutput")
    cand_d = nc.dram_tensor("cand", [ntok, E * A], f32, kind="ExternalOutput")
    probs_d = nc.dram_tensor("probs", [ntok, E], f32, kind="ExternalOutput")
    rtg_d = nc.dram_tensor("rtg", [ntok, 1], f32, kind="ExternalOutput")

    with tile.TileContext(nc) as tc:
        with tc.tile_pool(name="wp", bufs=1) as wp, \
             tc.tile_pool(name="xnat", bufs=5) as xnat_p, \
             tc.tile_pool(name="xtp", bufs=8) as xt_p, \
             tc.tile_pool(name="gp", bufs=2) as g_p, \
             tc.tile_pool(name="mp", bufs=2) as mp, \
             tc.tile_pool(name="ps_t", bufs=2, space="PSUM") as ps_t, \
             tc.tile_pool(name="ps_g", bufs=2, space="PSUM") as ps_g, \
             tc.tile_pool(name="ps_s", bufs=2, space="PSUM") as ps_s:

            # ---------------- constants / weights (resident) ----------------
            # memset/affine_select can't write f32r (ISA check), and f32r
            # matmul operands must be produced as f32r -> build constants in
            # f32 scratch, then DMA-bitcast into the f32r tiles.
            identf = wp.tile([128, 128], f32)
            make_identity(nc, identf[:])
            ident = wp.tile([128, 128], f32r)
            nc.sync.dma_start(ident[:], identf[:].bitcast(f32r))
            id32f = wp.tile([32, 32], f32)
            make_identity(nc, id32f[:])
            id32 = wp.tile([32, 32], f32r)
            nc.sync.dma_start(id32[:], id32f[:].bitcast(f32r))

            # prob-broadcast selectors: SE[:, 128e:128(e+1)] is [8,128] with
            # row e all-ones -> SE_e.T @ probsA broadcasts probsA row e onto
            # 128 partitions. (Compute engines can't address sub-32 partition
            # starts -> rows are written with SBUF->SBUF DMAs.)
            SEf = wp.tile([8, 128 * E], f32)
            onesf = wp.tile([1, TB], f32)
            nc.gpsimd.memset(onesf[:], 1.0)
            nc.gpsimd.memset(SEf[:], 0.0)
            for e in range(E):
                nc.sync.dma_start(SEf[e:e + 1, 128 * e:128 * (e + 1)],
                                  onesf[:, 0:128])
            SE = wp.tile([8, 128 * E], f32r)
            nc.sync.dma_start(SE[:], SEf[:].bitcast(f32r))

            ones8 = wp.tile([8, 1], f32r)
            nc.sync.dma_start(ones8[:], onesf[:, 0:8].bitcast(f32r))
            ones1x8 = wp.tile([1, 8], f32r)
            nc.sync.dma_start(ones1x8[:], onesf[:, 0:8].bitcast(f32r))
            onesrow = wp.tile([1, TB], f32r)
            nc.sync.dma_start(onesrow[:], onesf[:].bitcast(f32r))

            # candidate-action scale row, repeated per expert: [128, E*A]
            scale_t = wp.tile([128, E * A], f32)
            for e in range(E):
                nc.gpsimd.memset(scale_t[:, A * e:A * (e + 1)],
                                 float(SCALING[e]))

            # first-layer weights organized per feature-tile PAIR, DMA'd in
            # consumption order (router/value pair first) so the first L1
            # matmuls only wait on ~1MB, not the full 10.5MB.
            # wpair[p][:, 256k + 128j : 256k + 128(j+1)] = k-slice of ft 2p+j.
            # pair index: p = 0..7 experts, 8 = shared, 9 = router|value
            wpair = [wp.tile([128, 2048], f32r, tag=f"wpair{p}",
                             name=f"wpair{p}") for p in range(10)]

            def wslice(p, k, j):
                return wpair[p][:, 256 * k + 128 * j:256 * k + 128 * (j + 1)]

            for p in [9, 0, 1, 2, 3, 4, 5, 6, 7, 8]:
                for k in range(8):
                    hs = slice(128 * k, 128 * (k + 1))
                    if p < 8:
                        nc.sync.dma_start(
                            wpair[p][:, 256 * k:256 * (k + 1)],
                            ew1_d[p, hs, :].bitcast(f32r))
                    elif p == 8:
                        nc.sync.dma_start(
                            wpair[8][:, 256 * k:256 * (k + 1)],
                            sw1_d[hs, :].bitcast(f32r))
                    else:
                        nc.sync.dma_start(wslice(9, k, 0),
                                          rw1_d[hs, :].bitcast(f32r))
                        nc.sync.dma_start(wslice(9, k, 1),
                                          vw1_d[hs, :].bitcast(f32r))

            # expert second-layer weights [128, 16*32]
            w2 = wp.tile([128, 512], f32r)
            if BF16_G:
                w2b = wp.tile([128, 512], gdt, name="w2b")
            for e in range(E):
                for j in range(2):
                    nc.sync.dma_start(
                        w2[:, 32 * (2 * e + j):32 * (2 * e + j + 1)],
                        ew2_d[e, 128 * j:128 * (j + 1), :].bitcast(f32r))
            if BF16_G:
                nc.vector.tensor_copy(w2b[:], w2[:].bitcast(f32))
            sw2t = wp.tile([128, 64], f32r)
            if BF16_G:
                sw2b = wp.tile([128, 64], gdt, name="sw2b")
            for j in range(2):
                nc.sync.dma_start(sw2t[:, 32 * j:32 * (j + 1)],
                                  sw2_d[128 * j:128 * (j + 1), :].bitcast(f32r))
            if BF16_G:
                nc.vector.tensor_copy(sw2b[:], sw2t[:].bitcast(f32))
            w2u = w2b if BF16_G else w2
            sw2u = sw2b if BF16_G else sw2t
            rw2t = wp.tile([128, E], f32r)
            nc.sync.dma_start(rw2t[:], rw2_d[:, :].bitcast(f32r))
            vw2t = wp.tile([128, 1], f32r)
            nc.sync.dma_start(vw2t[:], vw2_d[:, :].bitcast(f32r))
            mw1t = wp.tile([32, 128], f32r)
            nc.sync.dma_start(mw1t[:], mw1_d[:, :].bitcast(f32r))
            mw2t = wp.tile([128, 32], f32r)
            nc.sync.dma_start(mw2t[:], mw2_d[:, :].bitcast(f32r))
            eb2t = wp.tile([8, 32], f32r)
            nc.sync.dma_start(eb2t[:], eb2_d[:, :].bitcast(f32r))
            sb2r = wp.tile([1, 32], f32r)
            nc.sync.dma_start(sb2r[:], sb2_d[None, :].bitcast(f32r))
            mb2r = wp.tile([1, 32], f32r)
            nc.sync.dma_start(mb2r[:], mb2_d[None, :].bitcast(f32r))

            # first-layer biases as a row vector [1, 128] per feature tile
            # (accumulated into psum with a K=1 ones-matmul); value tile 19
            # keeps a column bias for the DVE relu instead.
            b1r = wp.tile([1, 128 * FT], f32r)
            for ftt in range(16):
                e, j = ftt // 2, ftt % 2
                nc.sync.dma_start(b1r[:, 128 * ftt:128 * (ftt + 1)],
                                  eb1_d[e, 128 * j:128 * (j + 1)][None, :]
                                  .bitcast(f32r))
            for j in range(2):
                nc.sync.dma_start(b1r[:, 128 * (16 + j):128 * (17 + j)],
                                  sb1_d[128 * j:128 * (j + 1)][None, :]
                                  .bitcast(f32r))
            nc.sync.dma_start(b1r[:, 128 * 18:128 * 19],
                              rb1_d[:][None, :].bitcast(f32r))
            rb2c = wp.tile([8, 1], f32)
            nc.sync.dma_start(rb2c[:], rb2_d[:][:, None])
            vb1c = wp.tile([128, 1], f32)
            nc.sync.dma_start(vb1c[:], vb1_d[:][:, None])
            vb2c = wp.tile([1, 1], f32)
            nc.sync.dma_start(vb2c[:], vb2_d[:][:, None])
            mb1c = wp.tile([128, 1], f32)
            nc.sync.dma_start(mb1c[:], mb1_d[:][:, None])
            nvb2 = wp.tile([1, 1], f32)
            nc.vector.tensor_scalar_mul(nvb2[:], vb2c[:], -1.0)

            # ---------------- per-block pipeline ----------------
            def l1pair(pair, xt, bias_a=True, bias_b=True):
                """first-layer matmuls for a feature-tile pair sharing one
                [128, 2*TB] psum (adjacent banks); biases folded in via K=1
                ones-matmuls so one activation op can cover the pair."""
                psg = ps_g.tile([128, 2 * TB], f32, tag="psg", name="psg")
                for j, use_bias in ((0, bias_a), (1, bias_b)):
                    half = psg[:, TB * j:TB * (j + 1)]
                    ftt = 2 * pair + j
                    if use_bias:
                        nc.tensor.matmul(half,
                                         b1r[:, 128 * ftt:128 * (ftt + 1)],
                                         onesrow[:], start=True, stop=False)
                    for k in range(8):
                        nc.tensor.matmul(half, wslice(pair, k, j), xt[k][:],
                                         start=(k == 0 and not use_bias),
                                         stop=(k == 7))
                return psg

            for b in range(nblk):
                tok0 = b * TB

                # X in natural layout, then PE-transpose to XT [h, tok]
                xn = []
                for s in range(4):
                    t = xnat_p.tile([128, H], f32r, tag="xn", name="xn")
                    nc.gpsimd.dma_start(
                        t[:], x_d[tok0 + 128 * s:tok0 + 128 * (s + 1), :]
                        .bitcast(f32r))
                    xn.append(t)
                xt = []
                for k in range(8):
                    pst = ps_t.tile([128, TB], f32r, tag="pst", name="pst")
                    for s in range(4):
                        nc.tensor.matmul(
                            pst[:, 128 * s:128 * (s + 1)],
                            xn[s][:, 128 * k:128 * (k + 1)], ident[:],
                            is_transpose=True,
                            start=(s == 0), stop=(s == 3))
                    t = xt_p.tile([128, TB], f32r, tag="xt", name="xt")
                    nc.vector.tensor_copy(t[:], pst[:])
                    xt.append(t)

                # router (ft 18, gelu+bias-mm) and value (ft 19, relu on DVE)
                psg_rv = l1pair(9, xt, bias_a=True, bias_b=False)
                g18 = g_p.tile([128, TB], f32r, tag="g18")
                nc.scalar.activation(g18[:], psg_rv[:, 0:TB], AF.Gelu)
                g19 = g_p.tile([128, TB], f32r, tag="g19")
                nc.vector.tensor_scalar(g19[:], psg_rv[:, TB:2 * TB],
                                        vb1c[:], 0.0, ALU.add, ALU.max)
                psr = ps_s.tile([8, TB], f32, tag="ps_small", name="psr")
                nc.tensor.matmul(psr[:], rw2t[:], g18[:], start=True, stop=True)
                expR = mp.tile([8, TB], f32r, tag="expR")
                nc.scalar.activation(expR[:], psr[:], AF.Exp, bias=rb2c[:])
                # value head: sigmoid(z) = 1/(1+exp(-z)) via the Exp table
                psv = ps_s.tile([1, TB], f32, tag="ps_small", name="psv")
                nc.tensor.matmul(psv[:], vw2t[:], g19[:], start=True, stop=True)
                ev = mp.tile([1, TB], f32, tag="ev", bufs=1)
                nc.scalar.activation(ev[:], psv[:], AF.Exp, bias=nvb2[:],
                                     scale=-1.0)
                dv = mp.tile([1, TB], f32, tag="dv", bufs=1)
                nc.vector.tensor_scalar_add(dv[:], ev[:], 1.0)
                vsig = mp.tile([1, TB], f32, tag="vsig", bufs=1)
                nc.vector.reciprocal_approx_fast(vsig[:], dv[:])
                stack = mp.tile([10, TB], f32r, tag="stack")
                nc.sync.dma_start(stack[8:9, :], vsig[:].bitcast(f32r))
                nc.sync.dma_start(stack[9:10, :],
                                  mask_d[tok0:tok0 + TB][None, :].bitcast(f32r))
                # 1/sum(exp) and normalized probs
                pss = ps_s.tile([1, TB], f32, tag="ps_small", name="pss")
                nc.tensor.matmul(pss[:], ones8[:], expR[:], start=True,
                                 stop=True)
                recf = mp.tile([1, TB], f32, tag="recf", bufs=1)
                nc.vector.reciprocal_approx_fast(recf[:], pss[:])
                recipS = mp.tile([1, TB], f32r, tag="recipS", bufs=1)
                nc.sync.dma_start(recipS[:], recf[:].bitcast(f32r))
                ps8 = ps_s.tile([8, TB], f32, tag="ps_small", name="ps8")
                nc.tensor.matmul(ps8[:], ones1x8[:], recipS[:], start=True,
                                 stop=True)
                nc.vector.tensor_mul(stack[0:8, :], expR[:], ps8[:])

                # probs/rtg/mask transposes early (stack rows 0-9 final here);
                # frees the block tail to just the fin path
                pos = []
                for s in range(4):
                    cols = slice(128 * s, 128 * (s + 1))
                    rows = slice(tok0 + 128 * s, tok0 + 128 * (s + 1))
                    pspo = ps_s.tile([128, 10], f32r, tag="ps_small",
                                     name="pspo")
                    nc.tensor.matmul(pspo[:], stack[:, cols],
                                     ident[0:10, 0:10], is_transpose=True,
                                     start=True, stop=True)
                    po = mp.tile([128, 10], f32, tag="po", bufs=5)
                    nc.vector.tensor_copy(po[:], pspo[:])
                    pos.append(po)
                    nc.sync.dma_start(probs_d[rows, :], po[:, 0:8])
                    nc.sync.dma_start(rtg_d[rows, :], po[:, 8:9])

                # experts: pre-scale hidden activations by probsA[e]
                # (broadcast via selector matmul), accumulate all expert
                # second-layer matmuls into one [32,TB] psum. The per-token
                # scalar commutes through the contraction, so this equals
                # sum_e probsA_e * (h1_e @ W2_e).
                pswe = ps_s.tile([32, TB], f32, tag="ps_small", name="pswe")
                for e in range(E):
                    psg2 = l1pair(e, xt)
                    g2 = g_p.tile([128, 2 * TB], gdt, tag="g2", name="g2", bufs=3)
                    nc.scalar.activation(g2[:], psg2[:], AF.Gelu)
                    pbps = ps_s.tile([128, TB], f32, tag="ps_small",
                                     name="pbps")
                    nc.tensor.matmul(pbps[:], SE[:, 128 * e:128 * (e + 1)],
                                     stack[0:8, :], start=True, stop=True)
                    gs = g_p.tile([128, 2 * TB], gdt, tag="gs", name="gs")
                    nc.vector.tensor_mul(
                        gs[:].rearrange("p (r n) -> p r n", r=2),
                        g2[:].rearrange("p (r n) -> p r n", r=2),
                        pbps[:].unsqueeze(1).broadcast_to([128, 2, TB]))
                    nc.tensor.matmul(pswe[:], w2u[:, 64 * e:64 * e + 32],
                                     gs[:, 0:TB], start=(e == 0), stop=False)
                    nc.tensor.matmul(pswe[:], w2u[:, 64 * e + 32:64 * e + 64],
                                     gs[:, TB:2 * TB], start=False, stop=False)
                nc.tensor.matmul(pswe[:], eb2t[:], stack[0:8, :], start=False,
                                 stop=True)
                wen = mp.tile([32, TB], f32r, tag="wen")
                nc.vector.tensor_copy(wen[:], pswe[:])

                # shared expert (bias via K=1 ones-matmul)
                psg2s = l1pair(8, xt)
                g2sh = g_p.tile([128, 2 * TB], gdt, tag="g2", name="g2sh", bufs=3)
                nc.scalar.activation(g2sh[:], psg2s[:], AF.Gelu)
                pssh = ps_s.tile([32, TB], f32, tag="ps_small", name="pssh")
                nc.tensor.matmul(pssh[:], sw2u[:, 0:32], g2sh[:, 0:TB],
                                 start=True, stop=False)
                nc.tensor.matmul(pssh[:], sw2u[:, 32:64], g2sh[:, TB:2 * TB],
                                 start=False, stop=False)
                nc.tensor.matmul(pssh[:], sb2r[:], onesrow[:], start=False,
                                 stop=True)

                # moe = shared + weighted_expert; residual MLP
                moe = mp.tile([32, TB], f32r, tag="moe")
                nc.vector.tensor_add(moe[:], pssh[:], wen[:])
                psr1 = ps_s.tile([128, TB], f32, tag="ps_small", name="psr1")
                nc.tensor.matmul(psr1[:], mw1t[:], moe[:], start=True,
                                 stop=True)
                r1 = g_p.tile([128, TB], f32r, tag="r1")
                nc.scalar.activation(r1[:], psr1[:], AF.Gelu, bias=mb1c[:])
                psr2 = ps_s.tile([32, TB], f32, tag="ps_small", name="psr2")
                nc.tensor.matmul(psr2[:], mw2t[:], r1[:], start=True,
                                 stop=False)
                nc.tensor.matmul(psr2[:], mb2r[:], onesrow[:], start=False,
                                 stop=True)
                fin = mp.tile([32, TB], f32r, tag="fin")
                nc.vector.tensor_add(fin[:], psr2[:], wen[:])

                # transpose outputs back to token-major, apply mask, store
                for s in range(4):
                    cols = slice(128 * s, 128 * (s + 1))
                    rows = slice(tok0 + 128 * s, tok0 + 128 * (s + 1))
                    psf = ps_s.tile([128, 32], f32r, tag="ps_small",
                                    name="psf")
                    nc.tensor.matmul(psf[:], fin[:, cols], id32[:],
                                     is_transpose=True, start=True, stop=True)
                    fo = mp.tile([128, 32], f32, tag="fo")
                    nc.vector.tensor_scalar_mul(fo[:], psf[:], pos[s][:, 9:10])
                    nc.sync.dma_start(fin_d[rows, :], fo[:])

                # candidate actions (independent path): one broadcast DVE mul
                for s in range(4):
                    rows = slice(tok0 + 128 * s, tok0 + 128 * (s + 1))
                    bt = mp.tile([128, A], f32, tag="bt")
                    nc.sync.dma_start(bt[:], ba_d[rows, :])
                    cs = mp.tile([128, E * A], f32, tag="cs", bufs=1)
                    nc.vector.tensor_mul(
                        cs[:].rearrange("p (e a) -> p e a", e=E),
                        bt[:].unsqueeze(1).broadcast_to([128, E, A]),
                        scale_t[:].rearrange("p (e a) -> p e a", e=E))
                    nc.sync.dma_start(cand_d[rows, :], cs[:])

    nc.compile()
    return nc


def _get_compiled(ntok=NTOK):
    if ntok not in _compiled:
        _compiled[ntok] = build_nc(ntok)
    return _compiled[ntok]


def _run(inputs, trace=False, tmpdir=None):
    from concourse.bass_utils import run_bass_kernel_spmd

    nc = _get_compiled()

    state_rep = np.ascontiguousarray(inputs["state_rep"], dtype=np.float32)
    base_action = np.ascontiguousarray(inputs["base_action"], dtype=np.float32)
    attention_mask = np.ascontiguousarray(inputs["attention_mask"],
                                          dtype=np.float32)
    wmap = {k: np.ascontiguousarray(inputs[k], dtype=np.float32)
            for k in ("ew1", "eb1", "ew2", "eb2", "sw1", "sb1", "sw2", "sb2",
                      "rw1", "rb1", "rw2", "rb2", "vw1", "vb1", "vw2", "vb2",
                      "mw1", "mb1", "mw2", "mb2")}
    wmap["vw2"] = wmap["vw2"].reshape(RH, 1)
    wmap["vb2"] = wmap["vb2"].reshape(1)

    bpc = B // NCORES  # batches per core
    in_maps = []
    for c in range(NCORES):
        bs = slice(bpc * c, bpc * (c + 1))
        m = dict(wmap)
        m["x"] = state_rep[bs].reshape(NTOK, H)
        m["ba"] = base_action[bs].reshape(NTOK, A)
        m["mask"] = attention_mask[bs].reshape(NTOK)
        in_maps.append(m)

    res = run_bass_kernel_spmd(nc, in_maps, list(range(NCORES)),
                               trace=trace, tmpdir=tmpdir)

    fin = np.concatenate([res.results[c]["fin"] for c in range(NCORES)])
    cand = np.concatenate([res.results[c]["cand"] for c in range(NCORES)])
    probs = np.concatenate([res.results[c]["probs"] for c in range(NCORES)])
    rtg = np.concatenate([res.results[c]["rtg"] for c in range(NCORES)])

    out = (fin.reshape(B, S, A), cand.reshape(B, S, E, A),
           probs.reshape(B, S, E), rtg.reshape(B, S, 1))
    return out, res


def kernel(state_rep, base_action, attention_mask,
           sw1, sb1, sw2, sb2, ew1, eb1, ew2, eb2,
           rw1, rb1, rw2, rb2, mw1, mb1, mw2, mb2,
           vw1, vb1, vw2, vb2):
    out, _ = _run(dict(
        state_rep=state_rep, base_action=base_action,
        attention_mask=attention_mask,
        sw1=sw1, sb1=sb1, sw2=sw2, sb2=sb2, ew1=ew1, eb1=eb1, ew2=ew2,
        eb2=eb2, rw1=rw1, rb1=rb1, rw2=rw2, rb2=rb2, mw1=mw1, mb1=mb1,
        mw2=mw2, mb2=mb2, vw1=vw1, vb1=vb1, vw2=vw2, vb2=vb2))
    return out


# revision 24
# speedup vs baseline: 1.0331x; 1.0331x over previous
"""ActionMoE Trainium2 kernel.

Contract: kernel(**inputs) takes the FULL unsharded inputs (numpy arrays,
keyed as in setup_inputs()) and returns the full outputs
(final_action, candidate_actions, selection_probs, return_rtg).

Strategy: pure data parallelism over the batch dim (16 batches -> 2 per core,
8 cores, no collectives). Each core runs an identical NEFF over its 4096
tokens. On-chip layout is feature-major ("layout A": features on SBUF/PSUM
partitions, tokens on the free axis), processed in blocks of 512 tokens:

  1. PE-transpose X[tok,1024] -> XT[1024,tok]
  2. Fused first-layer matmul Xt @ Wcat with Wcat=[ew1|sw1|rw1|vw1] (f32r,
     1 cyc/row). Feature tiles are processed in PAIRS sharing one
     [128,1024] psum (2 banks) so one scalar-engine gelu covers both; the
     per-feature bias is accumulated into psum with a K=1 ones-matmul.
  3. Router: exp(logits+rb2) unnormalized; 1/sum via ones-matmul +
     fast-reciprocal; normalization folded into probsA = expR * bcast(1/sum).
  4. Experts: hidden activations pre-scaled by probsA[e] (broadcast onto 128
     partitions via a selector matmul, one DVE mul per expert), then ALL
     expert second-layer matmuls accumulate into a single [32,TB] psum.
  5. Shared expert / residual MLP as small matmuls, biases folded in as K=1
     ones-matmuls or activation bias. Value head: relu on DVE, sigmoid via
     the Exp table (avoids a Sigmoid ACT-table load) + fast reciprocal.
  6. PE-transpose results back to token-major; probs/rtg/mask ride one
     stacked [10,tok] transpose; mask applied token-major. Candidate
     actions are one broadcast DVE multiply per 128 tokens.
"""
import numpy as np

B, S, H, A, E, ED, RH = 16, 2048, 1024, 32, 8, 256, 128
NCORES = 8
NTOK = B * S // NCORES   # tokens per core
TB = 512                 # tokens per block
FT = 20                  # 2560/128 feature tiles: 0-15 experts, 16-17 shared, 18 router, 19 value
SCALING = np.linspace(0.8, 1.2, E, dtype=np.float32)
BF16_G = False  # expert/shared hidden activations + second-layer weights in bf16

_compiled = {}


def build_nc(ntok=NTOK):
    import concourse.tile as tile
    import concourse.mybir as mybir
    from concourse import bacc
    from concourse.masks import make_identity

    f32 = mybir.dt.float32
    f32r = mybir.dt.float32r
    bf16 = mybir.dt.bfloat16
    gdt = bf16 if BF16_G else f32r
    AF = mybir.ActivationFunctionType
    ALU = mybir.AluOpType
    nblk = ntok // TB

    nc = bacc.Bacc("TRN2", target_bir_lowering=False, debug=False,
                   num_devices=NCORES)

    x_d = nc.dram_tensor("x", [ntok, H], f32, kind="ExternalInput")
    ba_d = nc.dram_tensor("ba", [ntok, A], f32, kind="ExternalInput")
    mask_d = nc.dram_tensor("mask", [ntok], f32, kind="ExternalInput")
    ew1_d = nc.dram_tensor("ew1", [E, H, ED], f32, kind="ExternalInput")
    eb1_d = nc.dram_tensor("eb1", [E, ED], f32, kind="ExternalInput")
    ew2_d = nc.dram_tensor("ew2", [E, ED, A], f32, kind="ExternalInput")
    eb2_d = nc.dram_tensor("eb2", [E, A], f32, kind="ExternalInput")
    sw1_d = nc.dram_tensor("sw1", [H, ED], f32, kind="ExternalInput")
    sb1_d = nc.dram_tensor("sb1", [ED], f32, kind="ExternalInput")
    sw2_d = nc.dram_tensor("sw2", [ED, A], f32, kind="ExternalInput")
    sb2_d = nc.dram_tensor("sb2", [A], f32, kind="ExternalInput")
    rw1_d = nc.dram_tensor("rw1", [H, RH], f32, kind="ExternalInput")
    rb1_d = nc.dram_tensor("rb1", [RH], f32, kind="ExternalInput")
    rw2_d = nc.dram_tensor("rw2", [RH, E], f32, kind="ExternalInput")
    rb2_d = nc.dram_tensor("rb2", [E], f32, kind="ExternalInput")
    vw1_d = nc.dram_tensor("vw1", [H, RH], f32, kind="ExternalInput")
    vb1_d = nc.dram_tensor("vb1", [RH], f32, kind="ExternalInput")
    vw2_d = nc.dram_tensor("vw2", [RH, 1], f32, kind="ExternalInput")
    vb2_d = nc.dram_tensor("vb2", [1], f32, kind="ExternalInput")
    mw1_d = nc.dram_tensor("mw1", [A, RH], f32, kind="ExternalInput")
    mb1_d = nc.dram_tensor("mb1", [RH], f32, kind="ExternalInput")
    mw2_d = nc.dram_tensor("mw2", [RH, A], f32, kind="ExternalInput")
    mb2_d = nc.dram_tensor("mb2", [A], f32, kind="ExternalInput")

    fin_d = nc.dram_tensor("fin", [ntok, A], f32, kind="ExternalOutput")
    cand_d = nc.dram_tensor("cand", [ntok, E * A], f32, kind="ExternalOutput")
    probs_d = nc.dram_tensor("probs", [ntok, E], f32, kind="ExternalOutput")
    rtg_d = nc.dram_tensor("rtg", [ntok, 1], f32, kind="ExternalOutput")

    with tile.TileContext(nc) as tc:
        with tc.tile_pool(name="wp", bufs=1) as wp, \
             tc.tile_pool(name="xnat", bufs=5) as xnat_p, \
             tc.tile_pool(name="xtp", bufs=8) as xt_p, \
             tc.tile_pool(name="gp", bufs=2) as g_p, \
             tc.tile_pool(name="mp", bufs=2) as mp, \
             tc.tile_pool(name="ps_t", bufs=2, space="PSUM") as ps_t, \
             tc.tile_pool(name="ps_g", bufs=2, space="PSUM") as ps_g, \
             tc.tile_pool(name="ps_s", bufs=2, space="PSUM") as ps_s:

            # ---------------- constants / weights (resident) ----------------
            # memset/affine_select can't write f32r (ISA check), and f32r
            # matmul operands must be produced as f32r -> build constants in
            # f32 scratch, then DMA-bitcast into the f32r tiles.
            identf = wp.tile([128, 128], f32)
            make_identity(nc, identf[:])
            ident = wp.tile([128, 128], f32r)
            nc.sync.dma_start(ident[:], identf[:].bitcast(f32r))
            id32f = wp.tile([32, 32], f32)
            make_identity(nc, id32f[:])
            id32 = wp.tile([32, 32], f32r)
            nc.sync.dma_start(id32[:], id32f[:].bitcast(f32r))

            # prob-broadcast selectors: SE[:, 128e:128(e+1)] is [8,128] with
            # row e all-ones -> SE_e.T @ probsA broadcasts probsA row e onto
            # 128 partitions. (Compute engines can't address sub-32 partition
            # starts -> rows are written with SBUF->SBUF DMAs.)
            SEf = wp.tile([8, 128 * E], f32)
            onesf = wp.tile([1, TB], f32)
            nc.gpsimd.memset(onesf[:], 1.0)
            nc.gpsimd.memset(SEf[:], 0.0)
            for e in range(E):
                nc.sync.dma_start(SEf[e:e + 1, 128 * e:128 * (e + 1)],
                                  onesf[:, 0:128])
            SE = wp.tile([8, 128 * E], f32r)
            nc.sync.dma_start(SE[:], SEf[:].bitcast(f32r))

            ones8 = wp.tile([8, 1], f32r)
            nc.sync.dma_start(ones8[:], onesf[:, 0:8].bitcast(f32r))
            ones1x8 = wp.tile([1, 8], f32r)
            nc.sync.dma_start(ones1x8[:], onesf[:, 0:8].bitcast(f32r))
            onesrow = wp.tile([1, TB], f32r)
            nc.sync.dma_start(onesrow[:], onesf[:].bitcast(f32r))

            # candidate-action scale row, repeated per expert: [128, E*A]
            scale_t = wp.tile([128, E * A], f32)
            for e in range(E):
                nc.gpsimd.memset(scale_t[:, A * e:A * (e + 1)],
                                 float(SCALING[e]))

            # first-layer weights organized per feature-tile PAIR, DMA'd in
            # consumption order (router/value pair first) so the first L1
            # matmuls only wait on ~1MB, not the full 10.5MB.
            # wpair[p][:, 256k + 128j : 256k + 128(j+1)] = k-slice of ft 2p+j.
            # pair index: p = 0..7 experts, 8 = shared, 9 = router|value
            wpair = [wp.tile([128, 2048], f32r, tag=f"wpair{p}",
                             name=f"wpair{p}") for p in range(10)]

            def wslice(p, k, j):
                return wpair[p][:, 256 * k + 128 * j:256 * k + 128 * (j + 1)]

            # one big DMA per pair ([128, 8, 256] view of the [1024,256]
            # weight), issuance spread across otherwise-idle engine queues
            issuers = [nc.sync, nc.scalar]
            rv_dst = wpair[9][:].rearrange("p (k c) -> p k c", k=8)
            nc.sync.dma_start(
                rv_dst[:, :, 0:128],
                rw1_d[:].rearrange("(k p) c -> p k c", p=128).bitcast(f32r))
            nc.scalar.dma_start(
                rv_dst[:, :, 128:256],
                vw1_d[:].rearrange("(k p) c -> p k c", p=128).bitcast(f32r))
            for p in range(9):
                dst = wpair[p][:].rearrange("p (k c) -> p k c", k=8)
                srcd = ew1_d[p] if p < 8 else sw1_d[:]
                issuers[p % 2].dma_start(
                    dst[:],
                    srcd.rearrange("(k p) c -> p k c", p=128).bitcast(f32r))

            # expert second-layer weights [128, 16*32]
            w2 = wp.tile([128, 512], f32r)
            if BF16_G:
                w2b = wp.tile([128, 512], gdt, name="w2b")
            for e in range(E):
                for j in range(2):
                    nc.sync.dma_start(
                        w2[:, 32 * (2 * e + j):32 * (2 * e + j + 1)],
                        ew2_d[e, 128 * j:128 * (j + 1), :].bitcast(f32r))
            if BF16_G:
                nc.vector.tensor_copy(w2b[:], w2[:].bitcast(f32))
            sw2t = wp.tile([128, 64], f32r)
            if BF16_G:
                sw2b = wp.tile([128, 64], gdt, name="sw2b")
            for j in range(2):
                nc.sync.dma_start(sw2t[:, 32 * j:32 * (j + 1)],
                                  sw2_d[128 * j:128 * (j + 1), :].bitcast(f32r))
            if BF16_G:
                nc.vector.tensor_copy(sw2b[:], sw2t[:].bitcast(f32))
            w2u = w2b if BF16_G else w2
            sw2u = sw2b if BF16_G else sw2t
            rw2t = wp.tile([128, E], f32r)
            nc.sync.dma_start(rw2t[:], rw2_d[:, :].bitcast(f32r))
            vw2t = wp.tile([128, 1], f32r)
            nc.sync.dma_start(vw2t[:], vw2_d[:, :].bitcast(f32r))
            mw1t = wp.tile([32, 128], f32r)
            nc.sync.dma_start(mw1t[:], mw1_d[:, :].bitcast(f32r))
            mw2t = wp.tile([128, 32], f32r)
            nc.sync.dma_start(mw2t[:], mw2_d[:, :].bitcast(f32r))
            eb2t = wp.tile([8, 32], f32r)
            nc.sync.dma_start(eb2t[:], eb2_d[:, :].bitcast(f32r))
            sb2r = wp.tile([1, 32], f32r)
            nc.sync.dma_start(sb2r[:], sb2_d[None, :].bitcast(f32r))
            mb2r = wp.tile([1, 32], f32r)
            nc.sync.dma_start(mb2r[:], mb2_d[None, :].bitcast(f32r))

            # first-layer biases as a row vector [1, 128] per feature tile
            # (accumulated into psum with a K=1 ones-matmul); value tile 19
            # keeps a column bias for the DVE relu instead.
            b1r = wp.tile([1, 128 * FT], f32r)
            for ftt in range(16):
                e, j = ftt // 2, ftt % 2
                nc.sync.dma_start(b1r[:, 128 * ftt:128 * (ftt + 1)],
                                  eb1_d[e, 128 * j:128 * (j + 1)][None, :]
                                  .bitcast(f32r))
            for j in range(2):
                nc.sync.dma_start(b1r[:, 128 * (16 + j):128 * (17 + j)],
                                  sb1_d[128 * j:128 * (j + 1)][None, :]
                                  .bitcast(f32r))
            nc.sync.dma_start(b1r[:, 128 * 18:128 * 19],
                              rb1_d[:][None, :].bitcast(f32r))
            rb2c = wp.tile([8, 1], f32)
            nc.sync.dma_start(rb2c[:], rb2_d[:][:, None])
            vb1c = wp.tile([128, 1], f32)
            nc.sync.dma_start(vb1c[:], vb1_d[:][:, None])
            vb2c = wp.tile([1, 1], f32)
            nc.sync.dma_start(vb2c[:], vb2_d[:][:, None])
            mb1c = wp.tile([128, 1], f32)
            nc.sync.dma_start(mb1c[:], mb1_d[:][:, None])
            nvb2 = wp.tile([1, 1], f32)
            nc.vector.tensor_scalar_mul(nvb2[:], vb2c[:], -1.0)

            # ---------------- per-block pipeline ----------------
            def l1pair(pair, xt, bias_a=True, bias_b=True):
                """first-layer matmuls for a feature-tile pair sharing one
                [128, 2*TB] psum (adjacent banks); biases folded in via K=1
                ones-matmuls so one activation op can cover the pair."""
                psg = ps_g.tile([128, 2 * TB], f32, tag="psg", name="psg")
                for j, use_bias in ((0, bias_a), (1, bias_b)):
                    half = psg[:, TB * j:TB * (j + 1)]
                    ftt = 2 * pair + j
                    if use_bias:
                        nc.tensor.matmul(half,
                                         b1r[:, 128 * ftt:128 * (ftt + 1)],
                                         onesrow[:], start=True, stop=False)
                    for k in range(8):
                        nc.tensor.matmul(half, wslice(pair, k, j), xt[k][:],
                                         start=(k == 0 and not use_bias),
                                         stop=(k == 7))
                return psg

            for b in range(nblk):
                tok0 = b * TB

                # X in natural layout, then PE-transpose to XT [h, tok]
                xn = []
                for s in range(4):
                    t = xnat_p.tile([128, H], f32r, tag="xn", name="xn")
                    nc.gpsimd.dma_start(
                        t[:], x_d[tok0 + 128 * s:tok0 + 128 * (s + 1), :]
                        .bitcast(f32r))
                    xn.append(t)
                xt = []
                for k in range(8):
                    pst = ps_t.tile([128, TB], f32r, tag="pst", name="pst")
                    for s in range(4):
                        nc.tensor.matmul(
                            pst[:, 128 * s:128 * (s + 1)],
                            xn[s][:, 128 * k:128 * (k + 1)], ident[:],
                            is_transpose=True,
                            start=(s == 0), stop=(s == 3))
                    t = xt_p.tile([128, TB], f32r, tag="xt", name="xt")
                    nc.vector.tensor_copy(t[:], pst[:])
                    xt.append(t)

                # router (ft 18, gelu+bias-mm) and value (ft 19, relu on DVE)
                psg_rv = l1pair(9, xt, bias_a=True, bias_b=False)
                g18 = g_p.tile([128, TB], f32r, tag="g18")
                nc.scalar.activation(g18[:], psg_rv[:, 0:TB], AF.Gelu)
                g19 = g_p.tile([128, TB], f32r, tag="g19")
                nc.vector.tensor_scalar(g19[:], psg_rv[:, TB:2 * TB],
                                        vb1c[:], 0.0, ALU.add, ALU.max)
                psr = ps_s.tile([8, TB], f32, tag="ps_small", name="psr")
                nc.tensor.matmul(psr[:], rw2t[:], g18[:], start=True, stop=True)
                expR = mp.tile([8, TB], f32r, tag="expR")
                nc.scalar.activation(expR[:], psr[:], AF.Exp, bias=rb2c[:])
                # value head: sigmoid(z) = 1/(1+exp(-z)) via the Exp table
                psv = ps_s.tile([1, TB], f32, tag="ps_small", name="psv")
                nc.tensor.matmul(psv[:], vw2t[:], g19[:], start=True, stop=True)
                ev = mp.tile([1, TB], f32, tag="ev", bufs=1)
                nc.scalar.activation(ev[:], psv[:], AF.Exp, bias=nvb2[:],
                                     scale=-1.0)
                dv = mp.tile([1, TB], f32, tag="dv", bufs=1)
                nc.vector.tensor_scalar_add(dv[:], ev[:], 1.0)
                vsig = mp.tile([1, TB], f32, tag="vsig", bufs=1)
                nc.vector.reciprocal_approx_fast(vsig[:], dv[:])
                stack = mp.tile([10, TB], f32r, tag="stack")
                nc.sync.dma_start(stack[8:9, :], vsig[:].bitcast(f32r))
                nc.sync.dma_start(stack[9:10, :],
                                  mask_d[tok0:tok0 + TB][None, :].bitcast(f32r))
                # 1/sum(exp) and normalized probs
                pss = ps_s.tile([1, TB], f32, tag="ps_small", name="pss")
                nc.tensor.matmul(pss[:], ones8[:], expR[:], start=True,
                                 stop=True)
                recf = mp.tile([1, TB], f32, tag="recf", bufs=1)
                nc.vector.reciprocal_approx_fast(recf[:], pss[:])
                recipS = mp.tile([1, TB], f32r, tag="recipS", bufs=1)
                nc.sync.dma_start(recipS[:], recf[:].bitcast(f32r))
                ps8 = ps_s.tile([8, TB], f32, tag="ps_small", name="ps8")
                nc.tensor.matmul(ps8[:], ones1x8[:], recipS[:], start=True,
                                 stop=True)
                nc.vector.tensor_mul(stack[0:8, :], expR[:], ps8[:])

                # probs/rtg/mask transposes early (stack rows 0-9 final here);
                # frees the block tail to just the fin path
                pos = []
                for s in range(4):
                    cols = slice(128 * s, 128 * (s + 1))
                    rows = slice(tok0 + 128 * s, tok0 + 128 * (s + 1))
                    pspo = ps_s.tile([128, 10], f32r, tag="ps_small",
                                     name="pspo")
                    nc.tensor.matmul(pspo[:], stack[:, cols],
                                     ident[0:10, 0:10], is_transpose=True,
                                     start=True, stop=True)
                    po = mp.tile([128, 10], f32, tag="po", bufs=5)
                    nc.vector.tensor_copy(po[:], pspo[:])
                    pos.append(po)
                    nc.sync.dma_start(probs_d[rows, :], po[:, 0:8])
                    nc.sync.dma_start(rtg_d[rows, :], po[:, 8:9])

                # experts: pre-scale hidden activations by probsA[e]
                # (broadcast via selector matmul), accumulate all expert
                # second-layer matmuls into one [32,TB] psum. The per-token
                # scalar commutes through the contraction, so this equals
                # sum_e probsA_e * (h1_e @ W2_e).
                pswe = ps_s.tile([32, TB], f32, tag="ps_small", name="pswe")
                for e in range(E):
                    psg2 = l1pair(e, xt)
                    g2 = g_p.tile([128, 2 * TB], gdt, tag="g2", name="g2", bufs=3)
                    nc.scalar.activation(g2[:], psg2[:], AF.Gelu)
                    pbps = ps_s.tile([128, TB], f32, tag="ps_small",
                                     name="pbps")
                    nc.tensor.matmul(pbps[:], SE[:, 128 * e:128 * (e + 1)],
                                     stack[0:8, :], start=True, stop=True)
                    gs = g_p.tile([128, 2 * TB], gdt, tag="gs", name="gs")
                    nc.vector.tensor_mul(
                        gs[:].rearrange("p (r n) -> p r n", r=2),
                        g2[:].rearrange("p (r n) -> p r n", r=2),
                        pbps[:].unsqueeze(1).broadcast_to([128, 2, TB]))
                    nc.tensor.matmul(pswe[:], w2u[:, 64 * e:64 * e + 32],
                                     gs[:, 0:TB], start=(e == 0), stop=False)
                    nc.tensor.matmul(pswe[:], w2u[:, 64 * e + 32:64 * e + 64],
                                     gs[:, TB:2 * TB], start=False, stop=False)
                nc.tensor.matmul(pswe[:], eb2t[:], stack[0:8, :], start=False,
                                 stop=True)
                wen = mp.tile([32, TB], f32r, tag="wen")
                nc.vector.tensor_copy(wen[:], pswe[:])

                # shared expert (bias via K=1 ones-matmul)
                psg2s = l1pair(8, xt)
                g2sh = g_p.tile([128, 2 * TB], gdt, tag="g2", name="g2sh", bufs=3)
                nc.scalar.activation(g2sh[:], psg2s[:], AF.Gelu)
                pssh = ps_s.tile([32, TB], f32, tag="ps_small", name="pssh")
                nc.tensor.matmul(pssh[:], sw2u[:, 0:32], g2sh[:, 0:TB],
                                 start=True, stop=False)
                nc.tensor.matmul(pssh[:], sw2u[:, 32:64], g2sh[:, TB:2 * TB],
                                 start=False, stop=False)
                nc.tensor.matmul(pssh[:], sb2r[:], onesrow[:], start=False,
                                 stop=True)

                # moe = shared + weighted_expert; residual MLP
                moe = mp.tile([32, TB], f32r, tag="moe")
                nc.vector.tensor_add(moe[:], pssh[:], wen[:])
                psr1 = ps_s.tile([128, TB], f32, tag="ps_small", name="psr1")
                nc.tensor.matmul(psr1[:], mw1t[:], moe[:], start=True,
                                 stop=True)
                r1 = g_p.tile([128, TB], f32r, tag="r1")
                nc.scalar.activation(r1[:], psr1[:], AF.Gelu, bias=mb1c[:])
                psr2 = ps_s.tile([32, TB], f32, tag="ps_small", name="psr2")
                nc.tensor.matmul(psr2[:], mw2t[:], r1[:], start=True,
                                 stop=False)
                nc.tensor.matmul(psr2[:], mb2r[:], onesrow[:], start=False,
                                 stop=True)
                fin = mp.tile([32, TB], f32r, tag="fin")
                nc.vector.tensor_add(fin[:], psr2[:], wen[:])

                # transpose outputs back to token-major, apply mask, store
                for s in range(4):
                    cols = slice(128 * s, 128 * (s + 1))
                    rows = slice(tok0 + 128 * s, tok0 + 128 * (s + 1))
                    psf = ps_s.tile([128, 32], f32r, tag="ps_small",
                                    name="psf")
                    nc.tensor.matmul(psf[:], fin[:, cols], id32[:],
                                     is_transpose=True, start=True, stop=True)
                    fo = mp.tile([128, 32], f32, tag="fo")
                    nc.vector.tensor_scalar_mul(fo[:], psf[:], pos[s][:, 9:10])
                    nc.sync.dma_start(fin_d[rows, :], fo[:])

                # candidate actions (independent path): one broadcast DVE mul
                for s in range(4):
                    rows = slice(tok0 + 128 * s, tok0 + 128 * (s + 1))
                    bt = mp.tile([128, A], f32, tag="bt")
                    nc.sync.dma_start(bt[:], ba_d[rows, :])
                    cs = mp.tile([128, E * A], f32, tag="cs", bufs=1)
                    nc.vector.tensor_mul(
                        cs[:].rearrange("p (e a) -> p e a", e=E),
                        bt[:].unsqueeze(1).broadcast_to([128, E, A]),
                        scale_t[:].rearrange("p (e a) -> p e a", e=E))
                    nc.sync.dma_start(cand_d[rows, :], cs[:])

    nc.compile()
    return nc


def _get_compiled(ntok=NTOK):
    if ntok not in _compiled:
        _compiled[ntok] = build_nc(ntok)
    return _compiled[ntok]


def _run(inputs, trace=False, tmpdir=None):
    from concourse.bass_utils import run_bass_kernel_spmd

    nc = _get_compiled()

    state_rep = np.ascontiguousarray(inputs["state_rep"], dtype=np.float32)
    base_action = np.ascontiguousarray(inputs["base_action"], dtype=np.float32)
    attention_mask = np.ascontiguousarray(inputs["attention_mask"],
                                          dtype=np.float32)
    wmap = {k: np.ascontiguousarray(inputs[k], dtype=np.float32)
            for k in ("ew1", "eb1", "ew2", "eb2", "sw1", "sb1", "sw2", "sb2",
                      "rw1", "rb1", "rw2", "rb2", "vw1", "vb1", "vw2", "vb2",
                      "mw1", "mb1", "mw2", "mb2")}
    wmap["vw2"] = wmap["vw2"].reshape(RH, 1)
    wmap["vb2"] = wmap["vb2"].reshape(1)

    bpc = B // NCORES  # batches per core
    in_maps = []
    for c in range(NCORES):
        bs = slice(bpc * c, bpc * (c + 1))
        m = dict(wmap)
        m["x"] = state_rep[bs].reshape(NTOK, H)
        m["ba"] = base_action[bs].reshape(NTOK, A)
        m["mask"] = attention_mask[bs].reshape(NTOK)
        in_maps.append(m)

    res = run_bass_kernel_spmd(nc, in_maps, list(range(NCORES)),
                               trace=trace, tmpdir=tmpdir)

    fin = np.concatenate([res.results[c]["fin"] for c in range(NCORES)])
    cand = np.concatenate([res.results[c]["cand"] for c in range(NCORES)])
    probs = np.concatenate([res.results[c]["probs"] for c in range(NCORES)])
    rtg = np.concatenate([res.results[c]["rtg"] for c in range(NCORES)])

    out = (fin.reshape(B, S, A), cand.reshape(B, S, E, A),
           probs.reshape(B, S, E), rtg.reshape(B, S, 1))
    return out, res


def kernel(state_rep, base_action, attention_mask,
           sw1, sb1, sw2, sb2, ew1, eb1, ew2, eb2,
           rw1, rb1, rw2, rb2, mw1, mb1, mw2, mb2,
           vw1, vb1, vw2, vb2):
    out, _ = _run(dict(
        state_rep=state_rep, base_action=base_action,
        attention_mask=attention_mask,
        sw1=sw1, sb1=sb1, sw2=sw2, sb2=sb2, ew1=ew1, eb1=eb1, ew2=ew2,
        eb2=eb2, rw1=rw1, rb1=rb1, rw2=rw2, rb2=rb2, mw1=mw1, mb1=mb1,
        mw2=mw2, mb2=mb2, vw1=vw1, vb1=vb1, vw2=vw2, vb2=vb2))
    return out


# revision 25
# speedup vs baseline: 1.0785x; 1.0439x over previous
"""ActionMoE Trainium2 kernel.

Contract: kernel(**inputs) takes the FULL unsharded inputs (numpy arrays,
keyed as in setup_inputs()) and returns the full outputs
(final_action, candidate_actions, selection_probs, return_rtg).

Strategy: pure data parallelism over the batch dim (16 batches -> 2 per core,
8 cores, no collectives). Each core runs an identical NEFF over its 4096
tokens. On-chip layout is feature-major ("layout A": features on SBUF/PSUM
partitions, tokens on the free axis), processed in blocks of 512 tokens:

  1. PE-transpose X[tok,1024] -> XT[1024,tok]
  2. Fused first-layer matmul Xt @ Wcat with Wcat=[ew1|sw1|rw1|vw1] (f32r,
     1 cyc/row). Feature tiles are processed in PAIRS sharing one
     [128,1024] psum (2 banks) so one scalar-engine gelu covers both; the
     per-feature bias is accumulated into psum with a K=1 ones-matmul.
  3. Router: exp(logits+rb2) unnormalized; 1/sum via ones-matmul +
     fast-reciprocal; normalization folded into probsA = expR * bcast(1/sum).
  4. Experts: hidden activations pre-scaled by probsA[e] (broadcast onto 128
     partitions via a selector matmul, one DVE mul per expert), then ALL
     expert second-layer matmuls accumulate into a single [32,TB] psum.
  5. Shared expert / residual MLP as small matmuls, biases folded in as K=1
     ones-matmuls or activation bias. Value head: relu on DVE, sigmoid via
     the Exp table (avoids a Sigmoid ACT-table load) + fast reciprocal.
  6. PE-transpose results back to token-major; probs/rtg/mask ride one
     stacked [10,tok] transpose; mask applied token-major. Candidate
     actions are one broadcast DVE multiply per 128 tokens.
"""
import numpy as np

B, S, H, A, E, ED, RH = 16, 2048, 1024, 32, 8, 256, 128
NCORES = 8
NTOK = B * S // NCORES   # tokens per core
TB = 512                 # tokens per block
FT = 20                  # 2560/128 feature tiles: 0-15 experts, 16-17 shared, 18 router, 19 value
SCALING = np.linspace(0.8, 1.2, E, dtype=np.float32)
BF16_G = False  # expert/shared hidden activations + second-layer weights in bf16

_compiled = {}


def build_nc(ntok=NTOK):
    import concourse.tile as tile
    import concourse.mybir as mybir
    from concourse import bacc
    from concourse.masks import make_identity

    f32 = mybir.dt.float32
    f32r = mybir.dt.float32r
    bf16 = mybir.dt.bfloat16
    gdt = bf16 if BF16_G else f32r
    AF = mybir.ActivationFunctionType
    ALU = mybir.AluOpType
    nblk = ntok // TB

    nc = bacc.Bacc("TRN2", target_bir_lowering=False, debug=False,
                   num_devices=NCORES)

    x_d = nc.dram_tensor("x", [ntok, H], f32, kind="ExternalInput")
    ba_d = nc.dram_tensor("ba", [ntok, A], f32, kind="ExternalInput")
    mask_d = nc.dram_tensor("mask", [ntok], f32, kind="ExternalInput")
    ew1_d = nc.dram_tensor("ew1", [E, H, ED], f32, kind="ExternalInput")
    eb1_d = nc.dram_tensor("eb1", [E, ED], f32, kind="ExternalInput")
    ew2_d = nc.dram_tensor("ew2", [E, ED, A], f32, kind="ExternalInput")
    eb2_d = nc.dram_tensor("eb2", [E, A], f32, kind="ExternalInput")
    sw1_d = nc.dram_tensor("sw1", [H, ED], f32, kind="ExternalInput")
    sb1_d = nc.dram_tensor("sb1", [ED], f32, kind="ExternalInput")
    sw2_d = nc.dram_tensor("sw2", [ED, A], f32, kind="ExternalInput")
    sb2_d = nc.dram_tensor("sb2", [A], f32, kind="ExternalInput")
    rw1_d = nc.dram_tensor("rw1", [H, RH], f32, kind="ExternalInput")
    rb1_d = nc.dram_tensor("rb1", [RH], f32, kind="ExternalInput")
    rw2_d = nc.dram_tensor("rw2", [RH, E], f32, kind="ExternalInput")
    rb2_d = nc.dram_tensor("rb2", [E], f32, kind="ExternalInput")
    vw1_d = nc.dram_tensor("vw1", [H, RH], f32, kind="ExternalInput")
    vb1_d = nc.dram_tensor("vb1", [RH], f32, kind="ExternalInput")
    vw2_d = nc.dram_tensor("vw2", [RH, 1], f32, kind="ExternalInput")
    vb2_d = nc.dram_tensor("vb2", [1], f32, kind="ExternalInput")
    mw1_d = nc.dram_tensor("mw1", [A, RH], f32, kind="ExternalInput")
    mb1_d = nc.dram_tensor("mb1", [RH], f32, kind="ExternalInput")
    mw2_d = nc.dram_tensor("mw2", [RH, A], f32, kind="ExternalInput")
    mb2_d = nc.dram_tensor("mb2", [A], f32, kind="ExternalInput")

    fin_d = nc.dram_tensor("fin", [ntok, A], f32, kind="ExternalOutput")
    cand_d = nc.dram_tensor("cand", [ntok, E * A], f32, kind="ExternalOutput")
    probs_d = nc.dram_tensor("probs", [ntok, E], f32, kind="ExternalOutput")
    rtg_d = nc.dram_tensor("rtg", [ntok, 1], f32, kind="ExternalOutput")

    with tile.TileContext(nc) as tc:
        with tc.tile_pool(name="wp", bufs=1) as wp, \
             tc.tile_pool(name="xnat", bufs=5) as xnat_p, \
             tc.tile_pool(name="xtp", bufs=8) as xt_p, \
             tc.tile_pool(name="gp", bufs=2) as g_p, \
             tc.tile_pool(name="mp", bufs=2) as mp, \
             tc.tile_pool(name="ps_t", bufs=2, space="PSUM") as ps_t, \
             tc.tile_pool(name="ps_g", bufs=2, space="PSUM") as ps_g, \
             tc.tile_pool(name="ps_s", bufs=2, space="PSUM") as ps_s:

            # ---------------- constants / weights (resident) ----------------
            # memset/affine_select can't write f32r (ISA check), and f32r
            # matmul operands must be produced as f32r -> build constants in
            # f32 scratch, then DMA-bitcast into the f32r tiles.
            identf = wp.tile([128, 128], f32)
            make_identity(nc, identf[:])
            ident = wp.tile([128, 128], f32r)
            nc.sync.dma_start(ident[:], identf[:].bitcast(f32r))
            id32f = wp.tile([32, 32], f32)
            make_identity(nc, id32f[:])
            id32 = wp.tile([32, 32], f32r)
            nc.sync.dma_start(id32[:], id32f[:].bitcast(f32r))

            # prob-broadcast selectors: SE[:, 128e:128(e+1)] is [8,128] with
            # row e all-ones -> SE_e.T @ probsA broadcasts probsA row e onto
            # 128 partitions. (Compute engines can't address sub-32 partition
            # starts -> rows are written with SBUF->SBUF DMAs.)
            SEf = wp.tile([8, 128 * E], f32)
            onesf = wp.tile([1, TB], f32)
            nc.gpsimd.memset(onesf[:], 1.0)
            nc.gpsimd.memset(SEf[:], 0.0)
            for e in range(E):
                nc.sync.dma_start(SEf[e:e + 1, 128 * e:128 * (e + 1)],
                                  onesf[:, 0:128])
            SE = wp.tile([8, 128 * E], f32r)
            nc.sync.dma_start(SE[:], SEf[:].bitcast(f32r))

            ones8 = wp.tile([8, 1], f32r)
            nc.sync.dma_start(ones8[:], onesf[:, 0:8].bitcast(f32r))
            ones1x8 = wp.tile([1, 8], f32r)
            nc.sync.dma_start(ones1x8[:], onesf[:, 0:8].bitcast(f32r))
            onesrow = wp.tile([1, TB], f32r)
            nc.sync.dma_start(onesrow[:], onesf[:].bitcast(f32r))

            # candidate-action scale row, repeated per expert: [128, E*A]
            scale_t = wp.tile([128, E * A], f32)
            for e in range(E):
                nc.gpsimd.memset(scale_t[:, A * e:A * (e + 1)],
                                 float(SCALING[e]))

            # first-layer weights organized per feature-tile PAIR, DMA'd in
            # consumption order (router/value pair first) so the first L1
            # matmuls only wait on ~1MB, not the full 10.5MB.
            # wpair[p][:, 256k + 128j : 256k + 128(j+1)] = k-slice of ft 2p+j.
            # pair index: p = 0..7 experts, 8 = shared, 9 = router|value
            wpair = [wp.tile([128, 2048], f32r, tag=f"wpair{p}",
                             name=f"wpair{p}") for p in range(10)]

            def wslice(p, k, j):
                return wpair[p][:, 256 * k + 128 * j:256 * k + 128 * (j + 1)]

            # one big DMA per pair ([128, 8, 256] view of the [1024,256]
            # weight), issuance spread across otherwise-idle engine queues
            issuers = [nc.sync, nc.scalar]
            rv_dst = wpair[9][:].rearrange("p (k c) -> p k c", k=8)
            nc.sync.dma_start(
                rv_dst[:, :, 0:128],
                rw1_d[:].rearrange("(k p) c -> p k c", p=128).bitcast(f32r))
            nc.scalar.dma_start(
                rv_dst[:, :, 128:256],
                vw1_d[:].rearrange("(k p) c -> p k c", p=128).bitcast(f32r))
            for p in range(9):
                dst = wpair[p][:].rearrange("p (k c) -> p k c", k=8)
                srcd = ew1_d[p] if p < 8 else sw1_d[:]
                issuers[p % 2].dma_start(
                    dst[:],
                    srcd.rearrange("(k p) c -> p k c", p=128).bitcast(f32r))

            # expert second-layer weights [128, 16*32]
            w2 = wp.tile([128, 512], f32r)
            if BF16_G:
                w2b = wp.tile([128, 512], gdt, name="w2b")
            for e in range(E):
                for j in range(2):
                    nc.sync.dma_start(
                        w2[:, 32 * (2 * e + j):32 * (2 * e + j + 1)],
                        ew2_d[e, 128 * j:128 * (j + 1), :].bitcast(f32r))
            if BF16_G:
                nc.vector.tensor_copy(w2b[:], w2[:].bitcast(f32))
            sw2t = wp.tile([128, 64], f32r)
            if BF16_G:
                sw2b = wp.tile([128, 64], gdt, name="sw2b")
            for j in range(2):
                nc.sync.dma_start(sw2t[:, 32 * j:32 * (j + 1)],
                                  sw2_d[128 * j:128 * (j + 1), :].bitcast(f32r))
            if BF16_G:
                nc.vector.tensor_copy(sw2b[:], sw2t[:].bitcast(f32))
            w2u = w2b if BF16_G else w2
            sw2u = sw2b if BF16_G else sw2t
            rw2t = wp.tile([128, E], f32r)
            nc.sync.dma_start(rw2t[:], rw2_d[:, :].bitcast(f32r))
            vw2t = wp.tile([128, 1], f32r)
            nc.sync.dma_start(vw2t[:], vw2_d[:, :].bitcast(f32r))
            mw1t = wp.tile([32, 128], f32r)
            nc.sync.dma_start(mw1t[:], mw1_d[:, :].bitcast(f32r))
            mw2t = wp.tile([128, 32], f32r)
            nc.sync.dma_start(mw2t[:], mw2_d[:, :].bitcast(f32r))
            eb2t = wp.tile([8, 32], f32r)
            nc.sync.dma_start(eb2t[:], eb2_d[:, :].bitcast(f32r))
            sb2r = wp.tile([1, 32], f32r)
            nc.sync.dma_start(sb2r[:], sb2_d[None, :].bitcast(f32r))
            mb2r = wp.tile([1, 32], f32r)
            nc.sync.dma_start(mb2r[:], mb2_d[None, :].bitcast(f32r))

            # first-layer biases as [128,1] columns, added into psum by DVE
            # (cheaper than K=1 PE matmuls; ACT gelu then runs bias-free on
            # the whole pair). Column ft of b1c = bias for feature tile ft.
            b1c = wp.tile([128, FT], f32)
            for ftt in range(16):
                e, j = ftt // 2, ftt % 2
                nc.sync.dma_start(b1c[:, ftt:ftt + 1],
                                  eb1_d[e, 128 * j:128 * (j + 1)][:, None])
            for j in range(2):
                nc.sync.dma_start(b1c[:, 16 + j:17 + j],
                                  sb1_d[128 * j:128 * (j + 1)][:, None])
            nc.sync.dma_start(b1c[:, 18:19], rb1_d[:][:, None])
            rb2c = wp.tile([8, 1], f32)
            nc.sync.dma_start(rb2c[:], rb2_d[:][:, None])
            vb1c = wp.tile([128, 1], f32)
            nc.sync.dma_start(vb1c[:], vb1_d[:][:, None])
            vb2c = wp.tile([1, 1], f32)
            nc.sync.dma_start(vb2c[:], vb2_d[:][:, None])
            mb1c = wp.tile([128, 1], f32)
            nc.sync.dma_start(mb1c[:], mb1_d[:][:, None])
            nvb2 = wp.tile([1, 1], f32)
            nc.vector.tensor_scalar_mul(nvb2[:], vb2c[:], -1.0)

            # ---------------- per-block pipeline ----------------
            def l1pair(pair, xt, bias_a=True, bias_b=True):
                """first-layer matmuls for a feature-tile pair sharing one
                [128, 2*TB] psum (adjacent banks); per-feature biases added
                in-place by DVE so one bias-free activation op covers the
                pair."""
                psg = ps_g.tile([128, 2 * TB], f32, tag="psg", name="psg")
                for j, use_bias in ((0, bias_a), (1, bias_b)):
                    half = psg[:, TB * j:TB * (j + 1)]
                    ftt = 2 * pair + j
                    for k in range(8):
                        nc.tensor.matmul(half, wslice(pair, k, j), xt[k][:],
                                         start=(k == 0), stop=(k == 7))
                    if use_bias:
                        nc.vector.tensor_scalar_add(half, half,
                                                    b1c[:, ftt:ftt + 1])
                return psg

            for b in range(nblk):
                tok0 = b * TB

                # X in natural layout, then PE-transpose to XT [h, tok]
                xn = []
                for s in range(4):
                    t = xnat_p.tile([128, H], f32r, tag="xn", name="xn")
                    nc.gpsimd.dma_start(
                        t[:], x_d[tok0 + 128 * s:tok0 + 128 * (s + 1), :]
                        .bitcast(f32r))
                    xn.append(t)
                xt = []
                for k in range(8):
                    pst = ps_t.tile([128, TB], f32r, tag="pst", name="pst")
                    for s in range(4):
                        nc.tensor.matmul(
                            pst[:, 128 * s:128 * (s + 1)],
                            xn[s][:, 128 * k:128 * (k + 1)], ident[:],
                            is_transpose=True,
                            start=(s == 0), stop=(s == 3))
                    t = xt_p.tile([128, TB], f32r, tag="xt", name="xt")
                    nc.vector.tensor_copy(t[:], pst[:])
                    xt.append(t)

                # router (ft 18, gelu+bias-mm) and value (ft 19, relu on DVE)
                psg_rv = l1pair(9, xt, bias_a=True, bias_b=False)
                g18 = g_p.tile([128, TB], f32r, tag="g18")
                nc.scalar.activation(g18[:], psg_rv[:, 0:TB], AF.Gelu)
                g19 = g_p.tile([128, TB], f32r, tag="g19")
                nc.vector.tensor_scalar(g19[:], psg_rv[:, TB:2 * TB],
                                        vb1c[:], 0.0, ALU.add, ALU.max)
                psr = ps_s.tile([8, TB], f32, tag="ps_small", name="psr")
                nc.tensor.matmul(psr[:], rw2t[:], g18[:], start=True, stop=True)
                expR = mp.tile([8, TB], f32r, tag="expR")
                nc.scalar.activation(expR[:], psr[:], AF.Exp, bias=rb2c[:])
                # value head: sigmoid(z) = 1/(1+exp(-z)) via the Exp table
                psv = ps_s.tile([1, TB], f32, tag="ps_small", name="psv")
                nc.tensor.matmul(psv[:], vw2t[:], g19[:], start=True, stop=True)
                ev = mp.tile([1, TB], f32, tag="ev", bufs=1)
                nc.scalar.activation(ev[:], psv[:], AF.Exp, bias=nvb2[:],
                                     scale=-1.0)
                dv = mp.tile([1, TB], f32, tag="dv", bufs=1)
                nc.vector.tensor_scalar_add(dv[:], ev[:], 1.0)
                vsig = mp.tile([1, TB], f32, tag="vsig", bufs=1)
                nc.vector.reciprocal_approx_fast(vsig[:], dv[:])
                stack = mp.tile([10, TB], f32r, tag="stack")
                nc.sync.dma_start(stack[8:9, :], vsig[:].bitcast(f32r))
                nc.sync.dma_start(stack[9:10, :],
                                  mask_d[tok0:tok0 + TB][None, :].bitcast(f32r))
                # 1/sum(exp) and normalized probs
                pss = ps_s.tile([1, TB], f32, tag="ps_small", name="pss")
                nc.tensor.matmul(pss[:], ones8[:], expR[:], start=True,
                                 stop=True)
                recf = mp.tile([1, TB], f32, tag="recf", bufs=1)
                nc.vector.reciprocal_approx_fast(recf[:], pss[:])
                recipS = mp.tile([1, TB], f32r, tag="recipS", bufs=1)
                nc.sync.dma_start(recipS[:], recf[:].bitcast(f32r))
                ps8 = ps_s.tile([8, TB], f32, tag="ps_small", name="ps8")
                nc.tensor.matmul(ps8[:], ones1x8[:], recipS[:], start=True,
                                 stop=True)
                nc.vector.tensor_mul(stack[0:8, :], expR[:], ps8[:])

                # probs/rtg/mask transposes early (stack rows 0-9 final here);
                # frees the block tail to just the fin path
                pos = []
                for s in range(4):
                    cols = slice(128 * s, 128 * (s + 1))
                    rows = slice(tok0 + 128 * s, tok0 + 128 * (s + 1))
                    pspo = ps_s.tile([128, 10], f32r, tag="ps_small",
                                     name="pspo")
                    nc.tensor.matmul(pspo[:], stack[:, cols],
                                     ident[0:10, 0:10], is_transpose=True,
                                     start=True, stop=True)
                    po = mp.tile([128, 10], f32, tag="po", bufs=5)
                    nc.vector.tensor_copy(po[:], pspo[:])
                    pos.append(po)
                    nc.sync.dma_start(probs_d[rows, :], po[:, 0:8])
                    nc.sync.dma_start(rtg_d[rows, :], po[:, 8:9])

                # experts: pre-scale hidden activations by probsA[e]
                # (broadcast via selector matmul), accumulate all expert
                # second-layer matmuls into one [32,TB] psum. The per-token
                # scalar commutes through the contraction, so this equals
                # sum_e probsA_e * (h1_e @ W2_e).
                pswe = ps_s.tile([32, TB], f32, tag="ps_small", name="pswe")
                for e in range(E):
                    psg2 = l1pair(e, xt)
                    g2 = g_p.tile([128, 2 * TB], gdt, tag="g2", name="g2", bufs=3)
                    nc.scalar.activation(g2[:], psg2[:], AF.Gelu)
                    pbps = ps_s.tile([128, TB], f32, tag="ps_small",
                                     name="pbps")
                    nc.tensor.matmul(pbps[:], SE[:, 128 * e:128 * (e + 1)],
                                     stack[0:8, :], start=True, stop=True)
                    gs = g_p.tile([128, 2 * TB], gdt, tag="gs", name="gs")
                    nc.vector.tensor_mul(
                        gs[:].rearrange("p (r n) -> p r n", r=2),
                        g2[:].rearrange("p (r n) -> p r n", r=2),
                        pbps[:].unsqueeze(1).broadcast_to([128, 2, TB]))
                    nc.tensor.matmul(pswe[:], w2u[:, 64 * e:64 * e + 32],
                                     gs[:, 0:TB], start=(e == 0), stop=False)
                    nc.tensor.matmul(pswe[:], w2u[:, 64 * e + 32:64 * e + 64],
                                     gs[:, TB:2 * TB], start=False, stop=False)
                nc.tensor.matmul(pswe[:], eb2t[:], stack[0:8, :], start=False,
                                 stop=True)
                wen = mp.tile([32, TB], f32r, tag="wen")
                nc.vector.tensor_copy(wen[:], pswe[:])

                # shared expert (bias via K=1 ones-matmul)
                psg2s = l1pair(8, xt)
                g2sh = g_p.tile([128, 2 * TB], gdt, tag="g2", name="g2sh", bufs=3)
                nc.scalar.activation(g2sh[:], psg2s[:], AF.Gelu)
                pssh = ps_s.tile([32, TB], f32, tag="ps_small", name="pssh")
                nc.tensor.matmul(pssh[:], sw2u[:, 0:32], g2sh[:, 0:TB],
                                 start=True, stop=False)
                nc.tensor.matmul(pssh[:], sw2u[:, 32:64], g2sh[:, TB:2 * TB],
                                 start=False, stop=False)
                nc.tensor.matmul(pssh[:], sb2r[:], onesrow[:], start=False,
                                 stop=True)

                # moe = shared + weighted_expert; residual MLP
                moe = mp.tile([32, TB], f32r, tag="moe")
                nc.vector.tensor_add(moe[:], pssh[:], wen[:])
                psr1 = ps_s.tile([128, TB], f32, tag="ps_small", name="psr1")
                nc.tensor.matmul(psr1[:], mw1t[:], moe[:], start=True,
                                 stop=True)
                r1 = g_p.tile([128, TB], f32r, tag="r1")
                nc.scalar.activation(r1[:], psr1[:], AF.Gelu, bias=mb1c[:])
                psr2 = ps_s.tile([32, TB], f32, tag="ps_small", name="psr2")
                nc.tensor.matmul(psr2[:], mw2t[:], r1[:], start=True,
                                 stop=False)
                nc.tensor.matmul(psr2[:], mb2r[:], onesrow[:], start=False,
                                 stop=True)
                fin = mp.tile([32, TB], f32r, tag="fin")
                nc.vector.tensor_add(fin[:], psr2[:], wen[:])

                # transpose outputs back to token-major, apply mask, store
                for s in range(4):
                    cols = slice(128 * s, 128 * (s + 1))
                    rows = slice(tok0 + 128 * s, tok0 + 128 * (s + 1))
                    psf = ps_s.tile([128, 32], f32r, tag="ps_small",
                                    name="psf")
                    nc.tensor.matmul(psf[:], fin[:, cols], id32[:],
                                     is_transpose=True, start=True, stop=True)
                    fo = mp.tile([128, 32], f32, tag="fo")
                    nc.vector.tensor_scalar_mul(fo[:], psf[:], pos[s][:, 9:10])
                    nc.sync.dma_start(fin_d[rows, :], fo[:])

                # candidate actions (independent path): one broadcast DVE mul
                for s in range(4):
                    rows = slice(tok0 + 128 * s, tok0 + 128 * (s + 1))
                    bt = mp.tile([128, A], f32, tag="bt")
                    nc.sync.dma_start(bt[:], ba_d[rows, :])
                    cs = mp.tile([128, E * A], f32, tag="cs", bufs=1)
                    nc.vector.tensor_mul(
                        cs[:].rearrange("p (e a) -> p e a", e=E),
                        bt[:].unsqueeze(1).broadcast_to([128, E, A]),
                        scale_t[:].rearrange("p (e a) -> p e a", e=E))
                    nc.sync.dma_start(cand_d[rows, :], cs[:])

    nc.compile()
    return nc


def _get_compiled(ntok=NTOK):
    if ntok not in _compiled:
        _compiled[ntok] = build_nc(ntok)
    return _compiled[ntok]


def _run(inputs, trace=False, tmpdir=None):
    from concourse.bass_utils import run_bass_kernel_spmd

    nc = _get_compiled()

    state_rep = np.ascontiguousarray(inputs["state_rep"], dtype=np.float32)
    base_action = np.ascontiguousarray(inputs["base_action"], dtype=np.float32)
    attention_mask = np.ascontiguousarray(inputs["attention_mask"],
                                          dtype=np.float32)
    wmap = {k: np.ascontiguousarray(inputs[k], dtype=np.float32)
            for k in ("ew1", "eb1", "ew2", "eb2", "sw1", "sb1", "sw2", "sb2",
                      "rw1", "rb1", "rw2", "rb2", "vw1", "vb1", "vw2", "vb2",
                      "mw1", "mb1", "mw2", "mb2")}
    wmap["vw2"] = wmap["vw2"].reshape(RH, 1)
    wmap["vb2"] = wmap["vb2"].reshape(1)

    bpc = B // NCORES  # batches per core
    in_maps = []
    for c in range(NCORES):
        bs = slice(bpc * c, bpc * (c + 1))
        m = dict(wmap)
        m["x"] = state_rep[bs].reshape(NTOK, H)
        m["ba"] = base_action[bs].reshape(NTOK, A)
        m["mask"] = attention_mask[bs].reshape(NTOK)
        in_maps.append(m)

    res = run_bass_kernel_spmd(nc, in_maps, list(range(NCORES)),
                               trace=trace, tmpdir=tmpdir)

    fin = np.concatenate([res.results[c]["fin"] for c in range(NCORES)])
    cand = np.concatenate([res.results[c]["cand"] for c in range(NCORES)])
    probs = np.concatenate([res.results[c]["probs"] for c in range(NCORES)])
    rtg = np.concatenate([res.results[c]["rtg"] for c in range(NCORES)])

    out = (fin.reshape(B, S, A), cand.reshape(B, S, E, A),
           probs.reshape(B, S, E), rtg.reshape(B, S, 1))
    return out, res


def kernel(state_rep, base_action, attention_mask,
           sw1, sb1, sw2, sb2, ew1, eb1, ew2, eb2,
           rw1, rb1, rw2, rb2, mw1, mb1, mw2, mb2,
           vw1, vb1, vw2, vb2):
    out, _ = _run(dict(
        state_rep=state_rep, base_action=base_action,
        attention_mask=attention_mask,
        sw1=sw1, sb1=sb1, sw2=sw2, sb2=sb2, ew1=ew1, eb1=eb1, ew2=ew2,
        eb2=eb2, rw1=rw1, rb1=rb1, rw2=rw2, rb2=rb2, mw1=mw1, mb1=mb1,
        mw2=mw2, mb2=mb2, vw1=vw1, vb1=vb1, vw2=vw2, vb2=vb2))
    return out


# revision 26
# speedup vs baseline: 1.0880x; 1.0088x over previous
"""ActionMoE Trainium2 kernel.

Contract: kernel(**inputs) takes the FULL unsharded inputs (numpy arrays,
keyed as in setup_inputs()) and returns the full outputs
(final_action, candidate_actions, selection_probs, return_rtg).

Strategy: pure data parallelism over the batch dim (16 batches -> 2 per core,
8 cores, no collectives). Each core runs an identical NEFF over its 4096
tokens. On-chip layout is feature-major ("layout A": features on SBUF/PSUM
partitions, tokens on the free axis), processed in blocks of 512 tokens:

  1. PE-transpose X[tok,1024] -> XT[1024,tok]
  2. Fused first-layer matmul Xt @ Wcat with Wcat=[ew1|sw1|rw1|vw1] (f32r,
     1 cyc/row). Feature tiles are processed in PAIRS sharing one
     [128,1024] psum (2 banks) so one scalar-engine gelu covers both; the
     per-feature bias is accumulated into psum with a K=1 ones-matmul.
  3. Router: exp(logits+rb2) unnormalized; 1/sum via ones-matmul +
     fast-reciprocal; normalization folded into probsA = expR * bcast(1/sum).
  4. Experts: hidden activations pre-scaled by probsA[e] (broadcast onto 128
     partitions via a selector matmul, one DVE mul per expert), then ALL
     expert second-layer matmuls accumulate into a single [32,TB] psum.
  5. Shared expert / residual MLP as small matmuls, biases folded in as K=1
     ones-matmuls or activation bias. Value head: relu on DVE, sigmoid via
     the Exp table (avoids a Sigmoid ACT-table load) + fast reciprocal.
  6. PE-transpose results back to token-major; probs/rtg/mask ride one
     stacked [10,tok] transpose; mask applied token-major. Candidate
     actions are one broadcast DVE multiply per 128 tokens.
"""
import numpy as np

B, S, H, A, E, ED, RH = 16, 2048, 1024, 32, 8, 256, 128
NCORES = 8
NTOK = B * S // NCORES   # tokens per core
TB = 512                 # tokens per block
FT = 20                  # 2560/128 feature tiles: 0-15 experts, 16-17 shared, 18 router, 19 value
SCALING = np.linspace(0.8, 1.2, E, dtype=np.float32)
BF16_G = False  # expert/shared hidden activations + second-layer weights in bf16

_compiled = {}


def build_nc(ntok=NTOK):
    import concourse.tile as tile
    import concourse.mybir as mybir
    from concourse import bacc
    from concourse.masks import make_identity

    f32 = mybir.dt.float32
    f32r = mybir.dt.float32r
    bf16 = mybir.dt.bfloat16
    gdt = bf16 if BF16_G else f32r
    AF = mybir.ActivationFunctionType
    ALU = mybir.AluOpType
    nblk = ntok // TB

    nc = bacc.Bacc("TRN2", target_bir_lowering=False, debug=False,
                   num_devices=NCORES)

    x_d = nc.dram_tensor("x", [ntok, H], f32, kind="ExternalInput")
    ba_d = nc.dram_tensor("ba", [ntok, A], f32, kind="ExternalInput")
    mask_d = nc.dram_tensor("mask", [ntok], f32, kind="ExternalInput")
    ew1_d = nc.dram_tensor("ew1", [E, H, ED], f32, kind="ExternalInput")
    eb1_d = nc.dram_tensor("eb1", [E, ED], f32, kind="ExternalInput")
    ew2_d = nc.dram_tensor("ew2", [E, ED, A], f32, kind="ExternalInput")
    eb2_d = nc.dram_tensor("eb2", [E, A], f32, kind="ExternalInput")
    sw1_d = nc.dram_tensor("sw1", [H, ED], f32, kind="ExternalInput")
    sb1_d = nc.dram_tensor("sb1", [ED], f32, kind="ExternalInput")
    sw2_d = nc.dram_tensor("sw2", [ED, A], f32, kind="ExternalInput")
    sb2_d = nc.dram_tensor("sb2", [A], f32, kind="ExternalInput")
    rw1_d = nc.dram_tensor("rw1", [H, RH], f32, kind="ExternalInput")
    rb1_d = nc.dram_tensor("rb1", [RH], f32, kind="ExternalInput")
    rw2_d = nc.dram_tensor("rw2", [RH, E], f32, kind="ExternalInput")
    rb2_d = nc.dram_tensor("rb2", [E], f32, kind="ExternalInput")
    vw1_d = nc.dram_tensor("vw1", [H, RH], f32, kind="ExternalInput")
    vb1_d = nc.dram_tensor("vb1", [RH], f32, kind="ExternalInput")
    vw2_d = nc.dram_tensor("vw2", [RH, 1], f32, kind="ExternalInput")
    vb2_d = nc.dram_tensor("vb2", [1], f32, kind="ExternalInput")
    mw1_d = nc.dram_tensor("mw1", [A, RH], f32, kind="ExternalInput")
    mb1_d = nc.dram_tensor("mb1", [RH], f32, kind="ExternalInput")
    mw2_d = nc.dram_tensor("mw2", [RH, A], f32, kind="ExternalInput")
    mb2_d = nc.dram_tensor("mb2", [A], f32, kind="ExternalInput")

    fin_d = nc.dram_tensor("fin", [ntok, A], f32, kind="ExternalOutput")
    cand_d = nc.dram_tensor("cand", [ntok, E * A], f32, kind="ExternalOutput")
    probs_d = nc.dram_tensor("probs", [ntok, E], f32, kind="ExternalOutput")
    rtg_d = nc.dram_tensor("rtg", [ntok, 1], f32, kind="ExternalOutput")

    with tile.TileContext(nc) as tc:
        with tc.tile_pool(name="wp", bufs=1) as wp, \
             tc.tile_pool(name="xnat", bufs=5) as xnat_p, \
             tc.tile_pool(name="xtp", bufs=8) as xt_p, \
             tc.tile_pool(name="gp", bufs=2) as g_p, \
             tc.tile_pool(name="mp", bufs=2) as mp, \
             tc.tile_pool(name="ps_t", bufs=2, space="PSUM") as ps_t, \
             tc.tile_pool(name="ps_g", bufs=2, space="PSUM") as ps_g, \
             tc.tile_pool(name="ps_s", bufs=2, space="PSUM") as ps_s:

            # ---------------- constants / weights (resident) ----------------
            # memset/affine_select can't write f32r (ISA check), and f32r
            # matmul operands must be produced as f32r -> build constants in
            # f32 scratch, then DMA-bitcast into the f32r tiles.
            identf = wp.tile([128, 128], f32)
            make_identity(nc, identf[:])
            ident = wp.tile([128, 128], f32r)
            nc.sync.dma_start(ident[:], identf[:].bitcast(f32r))
            id32f = wp.tile([32, 32], f32)
            make_identity(nc, id32f[:])
            id32 = wp.tile([32, 32], f32r)
            nc.sync.dma_start(id32[:], id32f[:].bitcast(f32r))

            # prob-broadcast selectors: SE[:, 128e:128(e+1)] is [8,128] with
            # row e all-ones -> SE_e.T @ probsA broadcasts probsA row e onto
            # 128 partitions. (Compute engines can't address sub-32 partition
            # starts -> rows are written with SBUF->SBUF DMAs.)
            SEf = wp.tile([8, 128 * E], f32)
            onesf = wp.tile([1, TB], f32)
            nc.gpsimd.memset(onesf[:], 1.0)
            nc.gpsimd.memset(SEf[:], 0.0)
            for e in range(E):
                nc.sync.dma_start(SEf[e:e + 1, 128 * e:128 * (e + 1)],
                                  onesf[:, 0:128])
            SE = wp.tile([8, 128 * E], f32r)
            nc.sync.dma_start(SE[:], SEf[:].bitcast(f32r))

            ones8 = wp.tile([8, 1], f32r)
            nc.sync.dma_start(ones8[:], onesf[:, 0:8].bitcast(f32r))
            ones1x8 = wp.tile([1, 8], f32r)
            nc.sync.dma_start(ones1x8[:], onesf[:, 0:8].bitcast(f32r))
            onesrow = wp.tile([1, TB], f32r)
            nc.sync.dma_start(onesrow[:], onesf[:].bitcast(f32r))

            # candidate-action scale row, repeated per expert: [128, E*A]
            scale_t = wp.tile([128, E * A], f32)
            for e in range(E):
                nc.gpsimd.memset(scale_t[:, A * e:A * (e + 1)],
                                 float(SCALING[e]))

            # first-layer weights organized per feature-tile PAIR, DMA'd in
            # consumption order (router/value pair first) so the first L1
            # matmuls only wait on ~1MB, not the full 10.5MB.
            # wpair[p][:, 256k + 128j : 256k + 128(j+1)] = k-slice of ft 2p+j.
            # pair index: p = 0..7 experts, 8 = shared, 9 = router|value
            wpair = [wp.tile([128, 2048], f32r, tag=f"wpair{p}",
                             name=f"wpair{p}") for p in range(10)]

            def wslice(p, k, j):
                return wpair[p][:, 256 * k + 128 * j:256 * k + 128 * (j + 1)]

            # one big DMA per pair ([128, 8, 256] view of the [1024,256]
            # weight), issuance spread across otherwise-idle engine queues
            issuers = [nc.sync, nc.scalar, nc.gpsimd]
            rv_dst = wpair[9][:].rearrange("p (k c) -> p k c", k=8)
            nc.sync.dma_start(
                rv_dst[:, :, 0:128],
                rw1_d[:].rearrange("(k p) c -> p k c", p=128).bitcast(f32r))
            nc.scalar.dma_start(
                rv_dst[:, :, 128:256],
                vw1_d[:].rearrange("(k p) c -> p k c", p=128).bitcast(f32r))
            for p in range(9):
                dst = wpair[p][:].rearrange("p (k c) -> p k c", k=8)
                srcd = ew1_d[p] if p < 8 else sw1_d[:]
                issuers[p % 3].dma_start(
                    dst[:],
                    srcd.rearrange("(k p) c -> p k c", p=128).bitcast(f32r))

            # expert second-layer weights [128, 16*32]
            w2 = wp.tile([128, 512], f32r)
            if BF16_G:
                w2b = wp.tile([128, 512], gdt, name="w2b")
            for e in range(E):
                for j in range(2):
                    nc.sync.dma_start(
                        w2[:, 32 * (2 * e + j):32 * (2 * e + j + 1)],
                        ew2_d[e, 128 * j:128 * (j + 1), :].bitcast(f32r))
            if BF16_G:
                nc.vector.tensor_copy(w2b[:], w2[:].bitcast(f32))
            sw2t = wp.tile([128, 64], f32r)
            if BF16_G:
                sw2b = wp.tile([128, 64], gdt, name="sw2b")
            for j in range(2):
                nc.sync.dma_start(sw2t[:, 32 * j:32 * (j + 1)],
                                  sw2_d[128 * j:128 * (j + 1), :].bitcast(f32r))
            if BF16_G:
                nc.vector.tensor_copy(sw2b[:], sw2t[:].bitcast(f32))
            w2u = w2b if BF16_G else w2
            sw2u = sw2b if BF16_G else sw2t
            rw2t = wp.tile([128, E], f32r)
            nc.sync.dma_start(rw2t[:], rw2_d[:, :].bitcast(f32r))
            vw2t = wp.tile([128, 1], f32r)
            nc.sync.dma_start(vw2t[:], vw2_d[:, :].bitcast(f32r))
            mw1t = wp.tile([32, 128], f32r)
            nc.sync.dma_start(mw1t[:], mw1_d[:, :].bitcast(f32r))
            mw2t = wp.tile([128, 32], f32r)
            nc.sync.dma_start(mw2t[:], mw2_d[:, :].bitcast(f32r))
            eb2t = wp.tile([8, 32], f32r)
            nc.sync.dma_start(eb2t[:], eb2_d[:, :].bitcast(f32r))
            sb2r = wp.tile([1, 32], f32r)
            nc.sync.dma_start(sb2r[:], sb2_d[None, :].bitcast(f32r))
            mb2r = wp.tile([1, 32], f32r)
            nc.sync.dma_start(mb2r[:], mb2_d[None, :].bitcast(f32r))

            # first-layer biases as [128,1] columns, added into psum by DVE
            # (cheaper than K=1 PE matmuls; ACT gelu then runs bias-free on
            # the whole pair). Column ft of b1c = bias for feature tile ft.
            b1c = wp.tile([128, FT], f32)
            for ftt in range(16):
                e, j = ftt // 2, ftt % 2
                nc.sync.dma_start(b1c[:, ftt:ftt + 1],
                                  eb1_d[e, 128 * j:128 * (j + 1)][:, None])
            for j in range(2):
                nc.sync.dma_start(b1c[:, 16 + j:17 + j],
                                  sb1_d[128 * j:128 * (j + 1)][:, None])
            nc.sync.dma_start(b1c[:, 18:19], rb1_d[:][:, None])
            rb2c = wp.tile([8, 1], f32)
            nc.sync.dma_start(rb2c[:], rb2_d[:][:, None])
            vb1c = wp.tile([128, 1], f32)
            nc.sync.dma_start(vb1c[:], vb1_d[:][:, None])
            vb2c = wp.tile([1, 1], f32)
            nc.sync.dma_start(vb2c[:], vb2_d[:][:, None])
            mb1c = wp.tile([128, 1], f32)
            nc.sync.dma_start(mb1c[:], mb1_d[:][:, None])
            nvb2 = wp.tile([1, 1], f32)
            nc.vector.tensor_scalar_mul(nvb2[:], vb2c[:], -1.0)

            # ---------------- per-block pipeline ----------------
            def l1pair(pair, xt, bias_a=True, bias_b=True):
                """first-layer matmuls for a feature-tile pair sharing one
                [128, 2*TB] psum (adjacent banks); per-feature biases added
                in-place by DVE so one bias-free activation op covers the
                pair."""
                psg = ps_g.tile([128, 2 * TB], f32, tag="psg", name="psg")
                for j, use_bias in ((0, bias_a), (1, bias_b)):
                    half = psg[:, TB * j:TB * (j + 1)]
                    ftt = 2 * pair + j
                    for k in range(8):
                        nc.tensor.matmul(half, wslice(pair, k, j), xt[k][:],
                                         start=(k == 0), stop=(k == 7))
                    if use_bias:
                        nc.vector.tensor_scalar_add(half, half,
                                                    b1c[:, ftt:ftt + 1])
                return psg

            for b in range(nblk):
                tok0 = b * TB

                # X in natural layout, then PE-transpose to XT [h, tok]
                xn = []
                for s in range(4):
                    t = xnat_p.tile([128, H], f32r, tag="xn", name="xn")
                    nc.gpsimd.dma_start(
                        t[:], x_d[tok0 + 128 * s:tok0 + 128 * (s + 1), :]
                        .bitcast(f32r))
                    xn.append(t)
                xt = []
                for k in range(8):
                    pst = ps_t.tile([128, TB], f32r, tag="pst", name="pst")
                    for s in range(4):
                        nc.tensor.matmul(
                            pst[:, 128 * s:128 * (s + 1)],
                            xn[s][:, 128 * k:128 * (k + 1)], ident[:],
                            is_transpose=True,
                            start=(s == 0), stop=(s == 3))
                    t = xt_p.tile([128, TB], f32r, tag="xt", name="xt")
                    nc.vector.tensor_copy(t[:], pst[:])
                    xt.append(t)

                # router (ft 18, gelu+bias-mm) and value (ft 19, relu on DVE)
                psg_rv = l1pair(9, xt, bias_a=True, bias_b=False)
                g18 = g_p.tile([128, TB], f32r, tag="g18")
                nc.scalar.activation(g18[:], psg_rv[:, 0:TB], AF.Gelu)
                g19 = g_p.tile([128, TB], f32r, tag="g19")
                nc.vector.tensor_scalar(g19[:], psg_rv[:, TB:2 * TB],
                                        vb1c[:], 0.0, ALU.add, ALU.max)
                psr = ps_s.tile([8, TB], f32, tag="ps_small", name="psr")
                nc.tensor.matmul(psr[:], rw2t[:], g18[:], start=True, stop=True)
                expR = mp.tile([8, TB], f32r, tag="expR")
                nc.scalar.activation(expR[:], psr[:], AF.Exp, bias=rb2c[:])
                # value head: sigmoid(z) = 1/(1+exp(-z)) via the Exp table
                psv = ps_s.tile([1, TB], f32, tag="ps_small", name="psv")
                nc.tensor.matmul(psv[:], vw2t[:], g19[:], start=True, stop=True)
                ev = mp.tile([1, TB], f32, tag="ev", bufs=1)
                nc.scalar.activation(ev[:], psv[:], AF.Exp, bias=nvb2[:],
                                     scale=-1.0)
                dv = mp.tile([1, TB], f32, tag="dv", bufs=1)
                nc.vector.tensor_scalar_add(dv[:], ev[:], 1.0)
                vsig = mp.tile([1, TB], f32, tag="vsig", bufs=1)
                nc.vector.reciprocal_approx_fast(vsig[:], dv[:])
                stack = mp.tile([10, TB], f32r, tag="stack")
                nc.sync.dma_start(stack[8:9, :], vsig[:].bitcast(f32r))
                nc.sync.dma_start(stack[9:10, :],
                                  mask_d[tok0:tok0 + TB][None, :].bitcast(f32r))
                # 1/sum(exp) and normalized probs
                pss = ps_s.tile([1, TB], f32, tag="ps_small", name="pss")
                nc.tensor.matmul(pss[:], ones8[:], expR[:], start=True,
                                 stop=True)
                recf = mp.tile([1, TB], f32, tag="recf", bufs=1)
                nc.vector.reciprocal_approx_fast(recf[:], pss[:])
                recipS = mp.tile([1, TB], f32r, tag="recipS", bufs=1)
                nc.sync.dma_start(recipS[:], recf[:].bitcast(f32r))
                ps8 = ps_s.tile([8, TB], f32, tag="ps_small", name="ps8")
                nc.tensor.matmul(ps8[:], ones1x8[:], recipS[:], start=True,
                                 stop=True)
                nc.vector.tensor_mul(stack[0:8, :], expR[:], ps8[:])

                # probs/rtg/mask transposes early (stack rows 0-9 final here);
                # frees the block tail to just the fin path
                pos = []
                for s in range(4):
                    cols = slice(128 * s, 128 * (s + 1))
                    rows = slice(tok0 + 128 * s, tok0 + 128 * (s + 1))
                    pspo = ps_s.tile([128, 10], f32r, tag="ps_small",
                                     name="pspo")
                    nc.tensor.matmul(pspo[:], stack[:, cols],
                                     ident[0:10, 0:10], is_transpose=True,
                                     start=True, stop=True)
                    po = mp.tile([128, 10], f32, tag="po", bufs=5)
                    nc.vector.tensor_copy(po[:], pspo[:])
                    pos.append(po)
                    nc.sync.dma_start(probs_d[rows, :], po[:, 0:8])
                    nc.sync.dma_start(rtg_d[rows, :], po[:, 8:9])

                # experts: pre-scale hidden activations by probsA[e]
                # (broadcast via selector matmul), accumulate all expert
                # second-layer matmuls into one [32,TB] psum. The per-token
                # scalar commutes through the contraction, so this equals
                # sum_e probsA_e * (h1_e @ W2_e).
                pswe = ps_s.tile([32, TB], f32, tag="ps_small", name="pswe")
                for e in range(E):
                    psg2 = l1pair(e, xt)
                    g2 = g_p.tile([128, 2 * TB], gdt, tag="g2", name="g2", bufs=3)
                    nc.scalar.activation(g2[:], psg2[:], AF.Gelu)
                    pbps = ps_s.tile([128, TB], f32, tag="ps_small",
                                     name="pbps")
                    nc.tensor.matmul(pbps[:], SE[:, 128 * e:128 * (e + 1)],
                                     stack[0:8, :], start=True, stop=True)
                    gs = g_p.tile([128, 2 * TB], gdt, tag="gs", name="gs")
                    nc.vector.tensor_mul(
                        gs[:].rearrange("p (r n) -> p r n", r=2),
                        g2[:].rearrange("p (r n) -> p r n", r=2),
                        pbps[:].unsqueeze(1).broadcast_to([128, 2, TB]))
                    nc.tensor.matmul(pswe[:], w2u[:, 64 * e:64 * e + 32],
                                     gs[:, 0:TB], start=(e == 0), stop=False)
                    nc.tensor.matmul(pswe[:], w2u[:, 64 * e + 32:64 * e + 64],
                                     gs[:, TB:2 * TB], start=False, stop=False)
                nc.tensor.matmul(pswe[:], eb2t[:], stack[0:8, :], start=False,
                                 stop=True)
                wen = mp.tile([32, TB], f32r, tag="wen")
                nc.vector.tensor_copy(wen[:], pswe[:])

                # shared expert (bias via K=1 ones-matmul)
                psg2s = l1pair(8, xt)
                g2sh = g_p.tile([128, 2 * TB], gdt, tag="g2", name="g2sh", bufs=3)
                nc.scalar.activation(g2sh[:], psg2s[:], AF.Gelu)
                pssh = ps_s.tile([32, TB], f32, tag="ps_small", name="pssh")
                nc.tensor.matmul(pssh[:], sw2u[:, 0:32], g2sh[:, 0:TB],
                                 start=True, stop=False)
                nc.tensor.matmul(pssh[:], sw2u[:, 32:64], g2sh[:, TB:2 * TB],
                                 start=False, stop=False)
                nc.tensor.matmul(pssh[:], sb2r[:], onesrow[:], start=False,
                                 stop=True)

                # moe = shared + weighted_expert; residual MLP
                moe = mp.tile([32, TB], f32r, tag="moe")
                nc.vector.tensor_add(moe[:], pssh[:], wen[:])
                psr1 = ps_s.tile([128, TB], f32, tag="ps_small", name="psr1")
                nc.tensor.matmul(psr1[:], mw1t[:], moe[:], start=True,
                                 stop=True)
                r1 = g_p.tile([128, TB], f32r, tag="r1")
                nc.scalar.activation(r1[:], psr1[:], AF.Gelu, bias=mb1c[:])
                psr2 = ps_s.tile([32, TB], f32, tag="ps_small", name="psr2")
                nc.tensor.matmul(psr2[:], mw2t[:], r1[:], start=True,
                                 stop=False)
                nc.tensor.matmul(psr2[:], mb2r[:], onesrow[:], start=False,
                                 stop=True)
                fin = mp.tile([32, TB], f32r, tag="fin")
                nc.vector.tensor_add(fin[:], psr2[:], wen[:])

                # transpose outputs back to token-major, apply mask, store
                for s in range(4):
                    cols = slice(128 * s, 128 * (s + 1))
                    rows = slice(tok0 + 128 * s, tok0 + 128 * (s + 1))
                    psf = ps_s.tile([128, 32], f32r, tag="ps_small",
                                    name="psf")
                    nc.tensor.matmul(psf[:], fin[:, cols], id32[:],
                                     is_transpose=True, start=True, stop=True)
                    fo = mp.tile([128, 32], f32, tag="fo")
                    nc.vector.tensor_scalar_mul(fo[:], psf[:], pos[s][:, 9:10])
                    nc.sync.dma_start(fin_d[rows, :], fo[:])

                # candidate actions (independent path): one broadcast DVE mul
                for s in range(4):
                    rows = slice(tok0 + 128 * s, tok0 + 128 * (s + 1))
                    bt = mp.tile([128, A], f32, tag="bt")
                    nc.sync.dma_start(bt[:], ba_d[rows, :])
                    cs = mp.tile([128, E * A], f32, tag="cs", bufs=1)
                    nc.vector.tensor_mul(
                        cs[:].rearrange("p (e a) -> p e a", e=E),
                        bt[:].unsqueeze(1).broadcast_to([128, E, A]),
                        scale_t[:].rearrange("p (e a) -> p e a", e=E))
                    nc.sync.dma_start(cand_d[rows, :], cs[:])

    nc.compile()
    return nc


def _get_compiled(ntok=NTOK):
    if ntok not in _compiled:
        _compiled[ntok] = build_nc(ntok)
    return _compiled[ntok]


def _run(inputs, trace=False, tmpdir=None):
    from concourse.bass_utils import run_bass_kernel_spmd

    nc = _get_compiled()

    state_rep = np.ascontiguousarray(inputs["state_rep"], dtype=np.float32)
    base_action = np.ascontiguousarray(inputs["base_action"], dtype=np.float32)
    attention_mask = np.ascontiguousarray(inputs["attention_mask"],
                                          dtype=np.float32)
    wmap = {k: np.ascontiguousarray(inputs[k], dtype=np.float32)
            for k in ("ew1", "eb1", "ew2", "eb2", "sw1", "sb1", "sw2", "sb2",
                      "rw1", "rb1", "rw2", "rb2", "vw1", "vb1", "vw2", "vb2",
                      "mw1", "mb1", "mw2", "mb2")}
    wmap["vw2"] = wmap["vw2"].reshape(RH, 1)
    wmap["vb2"] = wmap["vb2"].reshape(1)

    bpc = B // NCORES  # batches per core
    in_maps = []
    for c in range(NCORES):
        bs = slice(bpc * c, bpc * (c + 1))
        m = dict(wmap)
        m["x"] = state_rep[bs].reshape(NTOK, H)
        m["ba"] = base_action[bs].reshape(NTOK, A)
        m["mask"] = attention_mask[bs].reshape(NTOK)
        in_maps.append(m)

    res = run_bass_kernel_spmd(nc, in_maps, list(range(NCORES)),
                               trace=trace, tmpdir=tmpdir)

    fin = np.concatenate([res.results[c]["fin"] for c in range(NCORES)])
    cand = np.concatenate([res.results[c]["cand"] for c in range(NCORES)])
    probs = np.concatenate([res.results[c]["probs"] for c in range(NCORES)])
    rtg = np.concatenate([res.results[c]["rtg"] for c in range(NCORES)])

    out = (fin.reshape(B, S, A), cand.reshape(B, S, E, A),
           probs.reshape(B, S, E), rtg.reshape(B, S, 1))
    return out, res


def kernel(state_rep, base_action, attention_mask,
           sw1, sb1, sw2, sb2, ew1, eb1, ew2, eb2,
           rw1, rb1, rw2, rb2, mw1, mb1, mw2, mb2,
           vw1, vb1, vw2, vb2):
    out, _ = _run(dict(
        state_rep=state_rep, base_action=base_action,
        attention_mask=attention_mask,
        sw1=sw1, sb1=sb1, sw2=sw2, sb2=sb2, ew1=ew1, eb1=eb1, ew2=ew2,
        eb2=eb2, rw1=rw1, rb1=rb1, rw2=rw2, rb2=rb2, mw1=mw1, mb1=mb1,
        mw2=mw2, mb2=mb2, vw1=vw1, vb1=vb1, vw2=vw2, vb2=vb2))
    return out


# revision 27
# speedup vs baseline: 1.1457x; 1.0530x over previous
"""ActionMoE Trainium2 kernel.

Contract: kernel(**inputs) takes the FULL unsharded inputs (numpy arrays,
keyed as in setup_inputs()) and returns the full outputs
(final_action, candidate_actions, selection_probs, return_rtg).

Strategy: pure data parallelism over the batch dim (16 batches -> 2 per core,
8 cores, no collectives). Each core runs an identical NEFF over its 4096
tokens. On-chip layout is feature-major ("layout A": features on SBUF/PSUM
partitions, tokens on the free axis), processed in blocks of 512 tokens:

  1. PE-transpose X[tok,1024] -> XT[1024,tok]
  2. Fused first-layer matmul Xt @ Wcat with Wcat=[ew1|sw1|rw1|vw1] (f32r,
     1 cyc/row). Feature tiles are processed in PAIRS sharing one
     [128,1024] psum (2 banks) so one scalar-engine gelu covers both; the
     per-feature bias is accumulated into psum with a K=1 ones-matmul.
  3. Router: exp(logits+rb2) unnormalized; 1/sum via ones-matmul +
     fast-reciprocal; normalization folded into probsA = expR * bcast(1/sum).
  4. Experts: hidden activations pre-scaled by probsA[e] (broadcast onto 128
     partitions via a selector matmul, one DVE mul per expert), then ALL
     expert second-layer matmuls accumulate into a single [32,TB] psum.
  5. Shared expert / residual MLP as small matmuls, biases folded in as K=1
     ones-matmuls or activation bias. Value head: relu on DVE, sigmoid via
     the Exp table (avoids a Sigmoid ACT-table load) + fast reciprocal.
  6. PE-transpose results back to token-major; probs/rtg/mask ride one
     stacked [10,tok] transpose; mask applied token-major. Candidate
     actions are one broadcast DVE multiply per 128 tokens.
"""
import numpy as np

B, S, H, A, E, ED, RH = 16, 2048, 1024, 32, 8, 256, 128
NCORES = 8
NTOK = B * S // NCORES   # tokens per core
TB = 512                 # tokens per block
FT = 20                  # 2560/128 feature tiles: 0-15 experts, 16-17 shared, 18 router, 19 value
SCALING = np.linspace(0.8, 1.2, E, dtype=np.float32)
BF16_G = False  # expert/shared hidden activations + second-layer weights in bf16

_compiled = {}


def build_nc(ntok=NTOK):
    import concourse.tile as tile
    import concourse.mybir as mybir
    from concourse import bacc
    from concourse.masks import make_identity

    f32 = mybir.dt.float32
    f32r = mybir.dt.float32r
    bf16 = mybir.dt.bfloat16
    gdt = bf16 if BF16_G else f32r
    AF = mybir.ActivationFunctionType
    ALU = mybir.AluOpType
    nblk = ntok // TB

    nc = bacc.Bacc("TRN2", target_bir_lowering=False, debug=False,
                   num_devices=NCORES)

    x_d = nc.dram_tensor("x", [ntok, H], f32, kind="ExternalInput")
    ba_d = nc.dram_tensor("ba", [ntok, A], f32, kind="ExternalInput")
    mask_d = nc.dram_tensor("mask", [ntok], f32, kind="ExternalInput")
    ew1_d = nc.dram_tensor("ew1", [E, H, ED], f32, kind="ExternalInput")
    eb1_d = nc.dram_tensor("eb1", [E, ED], f32, kind="ExternalInput")
    ew2_d = nc.dram_tensor("ew2", [E, ED, A], f32, kind="ExternalInput")
    eb2_d = nc.dram_tensor("eb2", [E, A], f32, kind="ExternalInput")
    sw1_d = nc.dram_tensor("sw1", [H, ED], f32, kind="ExternalInput")
    sb1_d = nc.dram_tensor("sb1", [ED], f32, kind="ExternalInput")
    sw2_d = nc.dram_tensor("sw2", [ED, A], f32, kind="ExternalInput")
    sb2_d = nc.dram_tensor("sb2", [A], f32, kind="ExternalInput")
    rw1_d = nc.dram_tensor("rw1", [H, RH], f32, kind="ExternalInput")
    rb1_d = nc.dram_tensor("rb1", [RH], f32, kind="ExternalInput")
    rw2_d = nc.dram_tensor("rw2", [RH, E], f32, kind="ExternalInput")
    rb2_d = nc.dram_tensor("rb2", [E], f32, kind="ExternalInput")
    vw1_d = nc.dram_tensor("vw1", [H, RH], f32, kind="ExternalInput")
    vb1_d = nc.dram_tensor("vb1", [RH], f32, kind="ExternalInput")
    vw2_d = nc.dram_tensor("vw2", [RH, 1], f32, kind="ExternalInput")
    vb2_d = nc.dram_tensor("vb2", [1], f32, kind="ExternalInput")
    mw1_d = nc.dram_tensor("mw1", [A, RH], f32, kind="ExternalInput")
    mb1_d = nc.dram_tensor("mb1", [RH], f32, kind="ExternalInput")
    mw2_d = nc.dram_tensor("mw2", [RH, A], f32, kind="ExternalInput")
    mb2_d = nc.dram_tensor("mb2", [A], f32, kind="ExternalInput")

    fin_d = nc.dram_tensor("fin", [ntok, A], f32, kind="ExternalOutput")
    cand_d = nc.dram_tensor("cand", [ntok, E * A], f32, kind="ExternalOutput")
    probs_d = nc.dram_tensor("probs", [ntok, E], f32, kind="ExternalOutput")
    rtg_d = nc.dram_tensor("rtg", [ntok, 1], f32, kind="ExternalOutput")

    with tile.TileContext(nc) as tc:
        with tc.tile_pool(name="wp", bufs=1) as wp, \
             tc.tile_pool(name="xnat", bufs=5) as xnat_p, \
             tc.tile_pool(name="xtp", bufs=12) as xt_p, \
             tc.tile_pool(name="gp", bufs=2) as g_p, \
             tc.tile_pool(name="mp", bufs=2) as mp, \
             tc.tile_pool(name="ps_t", bufs=2, space="PSUM") as ps_t, \
             tc.tile_pool(name="ps_g", bufs=2, space="PSUM") as ps_g, \
             tc.tile_pool(name="ps_s", bufs=2, space="PSUM") as ps_s:

            # ---------------- constants / weights (resident) ----------------
            # memset/affine_select can't write f32r (ISA check), and f32r
            # matmul operands must be produced as f32r -> build constants in
            # f32 scratch, then DMA-bitcast into the f32r tiles.
            identf = wp.tile([128, 128], f32)
            make_identity(nc, identf[:])
            ident = wp.tile([128, 128], f32r)
            nc.sync.dma_start(ident[:], identf[:].bitcast(f32r))
            id32f = wp.tile([32, 32], f32)
            make_identity(nc, id32f[:])
            id32 = wp.tile([32, 32], f32r)
            nc.sync.dma_start(id32[:], id32f[:].bitcast(f32r))

            # prob-broadcast selectors: SE[:, 128e:128(e+1)] is [8,128] with
            # row e all-ones -> SE_e.T @ probsA broadcasts probsA row e onto
            # 128 partitions. (Compute engines can't address sub-32 partition
            # starts -> rows are written with SBUF->SBUF DMAs.)
            SEf = wp.tile([8, 128 * E], f32)
            onesf = wp.tile([1, TB], f32)
            nc.gpsimd.memset(onesf[:], 1.0)
            nc.gpsimd.memset(SEf[:], 0.0)
            for e in range(E):
                nc.sync.dma_start(SEf[e:e + 1, 128 * e:128 * (e + 1)],
                                  onesf[:, 0:128])
            SE = wp.tile([8, 128 * E], f32r)
            nc.sync.dma_start(SE[:], SEf[:].bitcast(f32r))

            ones8 = wp.tile([8, 1], f32r)
            nc.sync.dma_start(ones8[:], onesf[:, 0:8].bitcast(f32r))
            ones1x8 = wp.tile([1, 8], f32r)
            nc.sync.dma_start(ones1x8[:], onesf[:, 0:8].bitcast(f32r))
            onesrow = wp.tile([1, TB], f32r)
            nc.sync.dma_start(onesrow[:], onesf[:].bitcast(f32r))

            # candidate-action scale row, repeated per expert: [128, E*A]
            scale_t = wp.tile([128, E * A], f32)
            for e in range(E):
                nc.gpsimd.memset(scale_t[:, A * e:A * (e + 1)],
                                 float(SCALING[e]))

            # prefetch block 0's X subtiles BEFORE the weight stream so the
            # transposes start immediately (gpsimd queue, ahead of weights)
            xn0 = []
            for s in range(4):
                t0_ = xnat_p.tile([128, H], f32r, tag="xn", name="xn")
                nc.gpsimd.dma_start(t0_[:],
                                    x_d[128 * s:128 * (s + 1), :].bitcast(f32r))
                xn0.append(t0_)

            # first-layer weights organized per feature-tile PAIR, DMA'd in
            # consumption order (router/value pair first) so the first L1
            # matmuls only wait on ~1MB, not the full 10.5MB.
            # wpair[p][:, 256k + 128j : 256k + 128(j+1)] = k-slice of ft 2p+j.
            # pair index: p = 0..7 experts, 8 = shared, 9 = router|value
            wpair = [wp.tile([128, 2048], f32r, tag=f"wpair{p}",
                             name=f"wpair{p}") for p in range(10)]

            def wslice(p, k, j):
                return wpair[p][:, 256 * k + 128 * j:256 * k + 128 * (j + 1)]

            # one big DMA per pair ([128, 8, 256] view of the [1024,256]
            # weight), issuance spread across otherwise-idle engine queues
            issuers = {0: nc.sync, 1: nc.scalar, 2: nc.sync,
                       3: nc.scalar, 4: nc.gpsimd, 5: nc.sync,
                       6: nc.scalar, 7: nc.gpsimd, 8: nc.gpsimd}
            rv_dst = wpair[9][:].rearrange("p (k c) -> p k c", k=8)
            nc.sync.dma_start(
                rv_dst[:, :, 0:128],
                rw1_d[:].rearrange("(k p) c -> p k c", p=128).bitcast(f32r))
            nc.scalar.dma_start(
                rv_dst[:, :, 128:256],
                vw1_d[:].rearrange("(k p) c -> p k c", p=128).bitcast(f32r))
            for p in range(9):
                dst = wpair[p][:].rearrange("p (k c) -> p k c", k=8)
                srcd = ew1_d[p] if p < 8 else sw1_d[:]
                issuers[p].dma_start(
                    dst[:],
                    srcd.rearrange("(k p) c -> p k c", p=128).bitcast(f32r))

            # expert second-layer weights [128, 16*32]
            w2 = wp.tile([128, 512], f32r)
            if BF16_G:
                w2b = wp.tile([128, 512], gdt, name="w2b")
            for e in range(E):
                for j in range(2):
                    nc.sync.dma_start(
                        w2[:, 32 * (2 * e + j):32 * (2 * e + j + 1)],
                        ew2_d[e, 128 * j:128 * (j + 1), :].bitcast(f32r))
            if BF16_G:
                nc.vector.tensor_copy(w2b[:], w2[:].bitcast(f32))
            sw2t = wp.tile([128, 64], f32r)
            if BF16_G:
                sw2b = wp.tile([128, 64], gdt, name="sw2b")
            for j in range(2):
                nc.sync.dma_start(sw2t[:, 32 * j:32 * (j + 1)],
                                  sw2_d[128 * j:128 * (j + 1), :].bitcast(f32r))
            if BF16_G:
                nc.vector.tensor_copy(sw2b[:], sw2t[:].bitcast(f32))
            w2u = w2b if BF16_G else w2
            sw2u = sw2b if BF16_G else sw2t
            rw2t = wp.tile([128, E], f32r)
            nc.sync.dma_start(rw2t[:], rw2_d[:, :].bitcast(f32r))
            vw2t = wp.tile([128, 1], f32r)
            nc.sync.dma_start(vw2t[:], vw2_d[:, :].bitcast(f32r))
            mw1t = wp.tile([32, 128], f32r)
            nc.sync.dma_start(mw1t[:], mw1_d[:, :].bitcast(f32r))
            mw2t = wp.tile([128, 32], f32r)
            nc.sync.dma_start(mw2t[:], mw2_d[:, :].bitcast(f32r))
            eb2t = wp.tile([8, 32], f32r)
            nc.sync.dma_start(eb2t[:], eb2_d[:, :].bitcast(f32r))
            sb2r = wp.tile([1, 32], f32r)
            nc.sync.dma_start(sb2r[:], sb2_d[None, :].bitcast(f32r))
            mb2r = wp.tile([1, 32], f32r)
            nc.sync.dma_start(mb2r[:], mb2_d[None, :].bitcast(f32r))

            # first-layer biases as [128,1] columns, added into psum by DVE
            # (cheaper than K=1 PE matmuls; ACT gelu then runs bias-free on
            # the whole pair). Column ft of b1c = bias for feature tile ft.
            b1c = wp.tile([128, FT], f32)
            for ftt in range(16):
                e, j = ftt // 2, ftt % 2
                nc.sync.dma_start(b1c[:, ftt:ftt + 1],
                                  eb1_d[e, 128 * j:128 * (j + 1)][:, None])
            for j in range(2):
                nc.sync.dma_start(b1c[:, 16 + j:17 + j],
                                  sb1_d[128 * j:128 * (j + 1)][:, None])
            nc.sync.dma_start(b1c[:, 18:19], rb1_d[:][:, None])
            rb2c = wp.tile([8, 1], f32)
            nc.sync.dma_start(rb2c[:], rb2_d[:][:, None])
            vb1c = wp.tile([128, 1], f32)
            nc.sync.dma_start(vb1c[:], vb1_d[:][:, None])
            vb2c = wp.tile([1, 1], f32)
            nc.sync.dma_start(vb2c[:], vb2_d[:][:, None])
            mb1c = wp.tile([128, 1], f32)
            nc.sync.dma_start(mb1c[:], mb1_d[:][:, None])
            nvb2 = wp.tile([1, 1], f32)
            nc.vector.tensor_scalar_mul(nvb2[:], vb2c[:], -1.0)

            # ---------------- per-block pipeline ----------------
            def l1pair(pair, xt, bias_a=True, bias_b=True):
                """first-layer matmuls for a feature-tile pair sharing one
                [128, 2*TB] psum (adjacent banks); per-feature biases added
                in-place by DVE so one bias-free activation op covers the
                pair."""
                psg = ps_g.tile([128, 2 * TB], f32, tag="psg", name="psg")
                for j, use_bias in ((0, bias_a), (1, bias_b)):
                    half = psg[:, TB * j:TB * (j + 1)]
                    ftt = 2 * pair + j
                    for k in range(8):
                        nc.tensor.matmul(half, wslice(pair, k, j), xt[k][:],
                                         start=(k == 0), stop=(k == 7))
                    if use_bias:
                        nc.vector.tensor_scalar_add(half, half,
                                                    b1c[:, ftt:ftt + 1])
                return psg

            for b in range(nblk):
                tok0 = b * TB

                # X in natural layout, then PE-transpose to XT [h, tok]
                if b == 0:
                    xn = xn0
                else:
                    xn = []
                    for s in range(4):
                        t = xnat_p.tile([128, H], f32r, tag="xn", name="xn")
                        nc.gpsimd.dma_start(
                            t[:], x_d[tok0 + 128 * s:tok0 + 128 * (s + 1), :]
                            .bitcast(f32r))
                        xn.append(t)
                xt = []
                for k in range(8):
                    pst = ps_t.tile([128, TB], f32r, tag="pst", name="pst")
                    for s in range(4):
                        nc.tensor.matmul(
                            pst[:, 128 * s:128 * (s + 1)],
                            xn[s][:, 128 * k:128 * (k + 1)], ident[:],
                            is_transpose=True,
                            start=(s == 0), stop=(s == 3))
                    t = xt_p.tile([128, TB], f32r, tag="xt", name="xt")
                    nc.vector.tensor_copy(t[:], pst[:])
                    xt.append(t)

                # router (ft 18, gelu+bias-mm) and value (ft 19, relu on DVE)
                psg_rv = l1pair(9, xt, bias_a=True, bias_b=False)
                g18 = g_p.tile([128, TB], f32r, tag="g18", bufs=1)
                nc.scalar.activation(g18[:], psg_rv[:, 0:TB], AF.Gelu)
                g19 = g_p.tile([128, TB], f32r, tag="g19", bufs=1)
                nc.vector.tensor_scalar(g19[:], psg_rv[:, TB:2 * TB],
                                        vb1c[:], 0.0, ALU.add, ALU.max)
                psr = ps_s.tile([8, TB], f32, tag="ps_small", name="psr")
                nc.tensor.matmul(psr[:], rw2t[:], g18[:], start=True, stop=True)
                expR = mp.tile([8, TB], f32r, tag="expR")
                nc.scalar.activation(expR[:], psr[:], AF.Exp, bias=rb2c[:])
                # value head: sigmoid(z) = 1/(1+exp(-z)) via the Exp table
                psv = ps_s.tile([1, TB], f32, tag="ps_small", name="psv")
                nc.tensor.matmul(psv[:], vw2t[:], g19[:], start=True, stop=True)
                ev = mp.tile([1, TB], f32, tag="ev", bufs=1)
                nc.scalar.activation(ev[:], psv[:], AF.Exp, bias=nvb2[:],
                                     scale=-1.0)
                dv = mp.tile([1, TB], f32, tag="dv", bufs=1)
                nc.vector.tensor_scalar_add(dv[:], ev[:], 1.0)
                vsig = mp.tile([1, TB], f32, tag="vsig", bufs=1)
                nc.vector.reciprocal_approx_fast(vsig[:], dv[:])
                stack = mp.tile([10, TB], f32r, tag="stack")
                nc.sync.dma_start(stack[8:9, :], vsig[:].bitcast(f32r))
                nc.sync.dma_start(stack[9:10, :],
                                  mask_d[tok0:tok0 + TB][None, :].bitcast(f32r))
                # 1/sum(exp) and normalized probs
                pss = ps_s.tile([1, TB], f32, tag="ps_small", name="pss")
                nc.tensor.matmul(pss[:], ones8[:], expR[:], start=True,
                                 stop=True)
                recf = mp.tile([1, TB], f32, tag="recf", bufs=1)
                nc.vector.reciprocal_approx_fast(recf[:], pss[:])
                recipS = mp.tile([1, TB], f32r, tag="recipS", bufs=1)
                nc.sync.dma_start(recipS[:], recf[:].bitcast(f32r))
                ps8 = ps_s.tile([8, TB], f32, tag="ps_small", name="ps8")
                nc.tensor.matmul(ps8[:], ones1x8[:], recipS[:], start=True,
                                 stop=True)
                nc.vector.tensor_mul(stack[0:8, :], expR[:], ps8[:])

                # probs/rtg/mask transposes early (stack rows 0-9 final here);
                # frees the block tail to just the fin path
                pos = []
                for s in range(4):
                    cols = slice(128 * s, 128 * (s + 1))
                    rows = slice(tok0 + 128 * s, tok0 + 128 * (s + 1))
                    pspo = ps_s.tile([128, 10], f32r, tag="ps_small",
                                     name="pspo")
                    nc.tensor.matmul(pspo[:], stack[:, cols],
                                     ident[0:10, 0:10], is_transpose=True,
                                     start=True, stop=True)
                    po = mp.tile([128, 10], f32, tag="po", bufs=5)
                    nc.vector.tensor_copy(po[:], pspo[:])
                    pos.append(po)
                    nc.sync.dma_start(probs_d[rows, :], po[:, 0:8])
                    nc.sync.dma_start(rtg_d[rows, :], po[:, 8:9])

                # experts: pre-scale hidden activations by probsA[e]
                # (broadcast via selector matmul), accumulate all expert
                # second-layer matmuls into one [32,TB] psum. The per-token
                # scalar commutes through the contraction, so this equals
                # sum_e probsA_e * (h1_e @ W2_e).
                pswe = ps_s.tile([32, TB], f32, tag="ps_small", name="pswe")
                for e in range(E):
                    psg2 = l1pair(e, xt)
                    g2 = g_p.tile([128, 2 * TB], gdt, tag="g2", name="g2", bufs=3)
                    nc.scalar.activation(g2[:], psg2[:], AF.Gelu)
                    pbps = ps_s.tile([128, TB], f32, tag="ps_small",
                                     name="pbps")
                    nc.tensor.matmul(pbps[:], SE[:, 128 * e:128 * (e + 1)],
                                     stack[0:8, :], start=True, stop=True)
                    gs = g_p.tile([128, 2 * TB], gdt, tag="gs", name="gs")
                    nc.vector.tensor_mul(
                        gs[:].rearrange("p (r n) -> p r n", r=2),
                        g2[:].rearrange("p (r n) -> p r n", r=2),
                        pbps[:].unsqueeze(1).broadcast_to([128, 2, TB]))
                    nc.tensor.matmul(pswe[:], w2u[:, 64 * e:64 * e + 32],
                                     gs[:, 0:TB], start=(e == 0), stop=False)
                    nc.tensor.matmul(pswe[:], w2u[:, 64 * e + 32:64 * e + 64],
                                     gs[:, TB:2 * TB], start=False, stop=False)
                nc.tensor.matmul(pswe[:], eb2t[:], stack[0:8, :], start=False,
                                 stop=True)
                wen = mp.tile([32, TB], f32r, tag="wen", bufs=1)
                nc.vector.tensor_copy(wen[:], pswe[:])

                # shared expert (bias via K=1 ones-matmul)
                psg2s = l1pair(8, xt)
                g2sh = g_p.tile([128, 2 * TB], gdt, tag="g2", name="g2sh", bufs=3)
                nc.scalar.activation(g2sh[:], psg2s[:], AF.Gelu)
                pssh = ps_s.tile([32, TB], f32, tag="ps_small", name="pssh")
                nc.tensor.matmul(pssh[:], sw2u[:, 0:32], g2sh[:, 0:TB],
                                 start=True, stop=False)
                nc.tensor.matmul(pssh[:], sw2u[:, 32:64], g2sh[:, TB:2 * TB],
                                 start=False, stop=False)
                nc.tensor.matmul(pssh[:], sb2r[:], onesrow[:], start=False,
                                 stop=True)

                # moe = shared + weighted_expert; residual MLP
                moe = mp.tile([32, TB], f32r, tag="moe", bufs=1)
                nc.vector.tensor_add(moe[:], pssh[:], wen[:])
                psr1 = ps_s.tile([128, TB], f32, tag="ps_small", name="psr1")
                nc.tensor.matmul(psr1[:], mw1t[:], moe[:], start=True,
                                 stop=True)
                r1 = g_p.tile([128, TB], f32r, tag="r1")
                nc.scalar.activation(r1[:], psr1[:], AF.Gelu, bias=mb1c[:])
                psr2 = ps_s.tile([32, TB], f32, tag="ps_small", name="psr2")
                nc.tensor.matmul(psr2[:], mw2t[:], r1[:], start=True,
                                 stop=False)
                nc.tensor.matmul(psr2[:], mb2r[:], onesrow[:], start=False,
                                 stop=True)
                fin = mp.tile([32, TB], f32r, tag="fin", bufs=1)
                nc.vector.tensor_add(fin[:], psr2[:], wen[:])

                # transpose outputs back to token-major, apply mask, store
                for s in range(4):
                    cols = slice(128 * s, 128 * (s + 1))
                    rows = slice(tok0 + 128 * s, tok0 + 128 * (s + 1))
                    psf = ps_s.tile([128, 32], f32r, tag="ps_small",
                                    name="psf")
                    nc.tensor.matmul(psf[:], fin[:, cols], id32[:],
                                     is_transpose=True, start=True, stop=True)
                    fo = mp.tile([128, 32], f32, tag="fo")
                    nc.vector.tensor_scalar_mul(fo[:], psf[:], pos[s][:, 9:10])
                    nc.sync.dma_start(fin_d[rows, :], fo[:])

                # candidate actions (independent path): one broadcast DVE mul
                for s in range(4):
                    rows = slice(tok0 + 128 * s, tok0 + 128 * (s + 1))
                    bt = mp.tile([128, A], f32, tag="bt")
                    nc.sync.dma_start(bt[:], ba_d[rows, :])
                    cs = mp.tile([128, E * A], f32, tag="cs", bufs=1)
                    nc.vector.tensor_mul(
                        cs[:].rearrange("p (e a) -> p e a", e=E),
                        bt[:].unsqueeze(1).broadcast_to([128, E, A]),
                        scale_t[:].rearrange("p (e a) -> p e a", e=E))
                    nc.sync.dma_start(cand_d[rows, :], cs[:])

    nc.compile()
    return nc


def _get_compiled(ntok=NTOK):
    if ntok not in _compiled:
        _compiled[ntok] = build_nc(ntok)
    return _compiled[ntok]


def _run(inputs, trace=False, tmpdir=None):
    from concourse.bass_utils import run_bass_kernel_spmd

    nc = _get_compiled()

    state_rep = np.ascontiguousarray(inputs["state_rep"], dtype=np.float32)
    base_action = np.ascontiguousarray(inputs["base_action"], dtype=np.float32)
    attention_mask = np.ascontiguousarray(inputs["attention_mask"],
                                          dtype=np.float32)
    wmap = {k: np.ascontiguousarray(inputs[k], dtype=np.float32)
            for k in ("ew1", "eb1", "ew2", "eb2", "sw1", "sb1", "sw2", "sb2",
                      "rw1", "rb1", "rw2", "rb2", "vw1", "vb1", "vw2", "vb2",
                      "mw1", "mb1", "mw2", "mb2")}
    wmap["vw2"] = wmap["vw2"].reshape(RH, 1)
    wmap["vb2"] = wmap["vb2"].reshape(1)

    bpc = B // NCORES  # batches per core
    in_maps = []
    for c in range(NCORES):
        bs = slice(bpc * c, bpc * (c + 1))
        m = dict(wmap)
        m["x"] = state_rep[bs].reshape(NTOK, H)
        m["ba"] = base_action[bs].reshape(NTOK, A)
        m["mask"] = attention_mask[bs].reshape(NTOK)
        in_maps.append(m)

    res = run_bass_kernel_spmd(nc, in_maps, list(range(NCORES)),
                               trace=trace, tmpdir=tmpdir)

    fin = np.concatenate([res.results[c]["fin"] for c in range(NCORES)])
    cand = np.concatenate([res.results[c]["cand"] for c in range(NCORES)])
    probs = np.concatenate([res.results[c]["probs"] for c in range(NCORES)])
    rtg = np.concatenate([res.results[c]["rtg"] for c in range(NCORES)])

    out = (fin.reshape(B, S, A), cand.reshape(B, S, E, A),
           probs.reshape(B, S, E), rtg.reshape(B, S, 1))
    return out, res


def kernel(state_rep, base_action, attention_mask,
           sw1, sb1, sw2, sb2, ew1, eb1, ew2, eb2,
           rw1, rb1, rw2, rb2, mw1, mb1, mw2, mb2,
           vw1, vb1, vw2, vb2):
    out, _ = _run(dict(
        state_rep=state_rep, base_action=base_action,
        attention_mask=attention_mask,
        sw1=sw1, sb1=sb1, sw2=sw2, sb2=sb2, ew1=ew1, eb1=eb1, ew2=ew2,
        eb2=eb2, rw1=rw1, rb1=rb1, rw2=rw2, rb2=rb2, mw1=mw1, mb1=mb1,
        mw2=mw2, mb2=mb2, vw1=vw1, vb1=vb1, vw2=vw2, vb2=vb2))
    return out


# revision 28
# speedup vs baseline: 1.1495x; 1.0033x over previous
"""ActionMoE Trainium2 kernel.

Contract: kernel(**inputs) takes the FULL unsharded inputs (numpy arrays,
keyed as in setup_inputs()) and returns the full outputs
(final_action, candidate_actions, selection_probs, return_rtg).

Strategy: pure data parallelism over the batch dim (16 batches -> 2 per core,
8 cores, no collectives). Each core runs an identical NEFF over its 4096
tokens. On-chip layout is feature-major ("layout A": features on SBUF/PSUM
partitions, tokens on the free axis), processed in blocks of 512 tokens:

  1. PE-transpose X[tok,1024] -> XT[1024,tok]
  2. Fused first-layer matmul Xt @ Wcat with Wcat=[ew1|sw1|rw1|vw1] (f32r,
     1 cyc/row). Feature tiles are processed in PAIRS sharing one
     [128,1024] psum (2 banks) so one scalar-engine gelu covers both; the
     per-feature bias is accumulated into psum with a K=1 ones-matmul.
  3. Router: exp(logits+rb2) unnormalized; 1/sum via ones-matmul +
     fast-reciprocal; normalization folded into probsA = expR * bcast(1/sum).
  4. Experts: hidden activations pre-scaled by probsA[e] (broadcast onto 128
     partitions via a selector matmul, one DVE mul per expert), then ALL
     expert second-layer matmuls accumulate into a single [32,TB] psum.
  5. Shared expert / residual MLP as small matmuls, biases folded in as K=1
     ones-matmuls or activation bias. Value head: relu on DVE, sigmoid via
     the Exp table (avoids a Sigmoid ACT-table load) + fast reciprocal.
  6. PE-transpose results back to token-major; probs/rtg/mask ride one
     stacked [10,tok] transpose; mask applied token-major. Candidate
     actions are one broadcast DVE multiply per 128 tokens.
"""
import numpy as np

B, S, H, A, E, ED, RH = 16, 2048, 1024, 32, 8, 256, 128
NCORES = 8
NTOK = B * S // NCORES   # tokens per core
TB = 512                 # tokens per block
FT = 20                  # 2560/128 feature tiles: 0-15 experts, 16-17 shared, 18 router, 19 value
SCALING = np.linspace(0.8, 1.2, E, dtype=np.float32)
BF16_G = False  # expert/shared hidden activations + second-layer weights in bf16

_compiled = {}


def build_nc(ntok=NTOK):
    import concourse.tile as tile
    import concourse.mybir as mybir
    from concourse import bacc
    from concourse.masks import make_identity

    f32 = mybir.dt.float32
    f32r = mybir.dt.float32r
    bf16 = mybir.dt.bfloat16
    gdt = bf16 if BF16_G else f32r
    AF = mybir.ActivationFunctionType
    ALU = mybir.AluOpType
    nblk = ntok // TB

    nc = bacc.Bacc("TRN2", target_bir_lowering=False, debug=False,
                   num_devices=NCORES)

    x_d = nc.dram_tensor("x", [ntok, H], f32, kind="ExternalInput")
    ba_d = nc.dram_tensor("ba", [ntok, A], f32, kind="ExternalInput")
    mask_d = nc.dram_tensor("mask", [ntok], f32, kind="ExternalInput")
    ew1_d = nc.dram_tensor("ew1", [E, H, ED], f32, kind="ExternalInput")
    eb1_d = nc.dram_tensor("eb1", [E, ED], f32, kind="ExternalInput")
    ew2_d = nc.dram_tensor("ew2", [E, ED, A], f32, kind="ExternalInput")
    eb2_d = nc.dram_tensor("eb2", [E, A], f32, kind="ExternalInput")
    sw1_d = nc.dram_tensor("sw1", [H, ED], f32, kind="ExternalInput")
    sb1_d = nc.dram_tensor("sb1", [ED], f32, kind="ExternalInput")
    sw2_d = nc.dram_tensor("sw2", [ED, A], f32, kind="ExternalInput")
    sb2_d = nc.dram_tensor("sb2", [A], f32, kind="ExternalInput")
    rw1_d = nc.dram_tensor("rw1", [H, RH], f32, kind="ExternalInput")
    rb1_d = nc.dram_tensor("rb1", [RH], f32, kind="ExternalInput")
    rw2_d = nc.dram_tensor("rw2", [RH, E], f32, kind="ExternalInput")
    rb2_d = nc.dram_tensor("rb2", [E], f32, kind="ExternalInput")
    vw1_d = nc.dram_tensor("vw1", [H, RH], f32, kind="ExternalInput")
    vb1_d = nc.dram_tensor("vb1", [RH], f32, kind="ExternalInput")
    vw2_d = nc.dram_tensor("vw2", [RH, 1], f32, kind="ExternalInput")
    vb2_d = nc.dram_tensor("vb2", [1], f32, kind="ExternalInput")
    mw1_d = nc.dram_tensor("mw1", [A, RH], f32, kind="ExternalInput")
    mb1_d = nc.dram_tensor("mb1", [RH], f32, kind="ExternalInput")
    mw2_d = nc.dram_tensor("mw2", [RH, A], f32, kind="ExternalInput")
    mb2_d = nc.dram_tensor("mb2", [A], f32, kind="ExternalInput")

    fin_d = nc.dram_tensor("fin", [ntok, A], f32, kind="ExternalOutput")
    cand_d = nc.dram_tensor("cand", [ntok, E * A], f32, kind="ExternalOutput")
    probs_d = nc.dram_tensor("probs", [ntok, E], f32, kind="ExternalOutput")
    rtg_d = nc.dram_tensor("rtg", [ntok, 1], f32, kind="ExternalOutput")

    with tile.TileContext(nc) as tc:
        with tc.tile_pool(name="wp", bufs=1) as wp, \
             tc.tile_pool(name="xnat", bufs=5) as xnat_p, \
             tc.tile_pool(name="xtp", bufs=12) as xt_p, \
             tc.tile_pool(name="gp", bufs=2) as g_p, \
             tc.tile_pool(name="mp", bufs=2) as mp, \
             tc.tile_pool(name="ps_t", bufs=2, space="PSUM") as ps_t, \
             tc.tile_pool(name="ps_g", bufs=2, space="PSUM") as ps_g, \
             tc.tile_pool(name="ps_s", bufs=2, space="PSUM") as ps_s:

            # ---------------- constants / weights (resident) ----------------
            # memset/affine_select can't write f32r (ISA check), and f32r
            # matmul operands must be produced as f32r -> build constants in
            # f32 scratch, then DMA-bitcast into the f32r tiles.
            identf = wp.tile([128, 128], f32)
            make_identity(nc, identf[:])
            ident = wp.tile([128, 128], f32r)
            nc.sync.dma_start(ident[:], identf[:].bitcast(f32r))
            id32f = wp.tile([32, 32], f32)
            make_identity(nc, id32f[:])
            id32 = wp.tile([32, 32], f32r)
            nc.sync.dma_start(id32[:], id32f[:].bitcast(f32r))

            # prob-broadcast selectors: SE[:, 128e:128(e+1)] is [8,128] with
            # row e all-ones -> SE_e.T @ probsA broadcasts probsA row e onto
            # 128 partitions. (Compute engines can't address sub-32 partition
            # starts -> rows are written with SBUF->SBUF DMAs.)
            SEf = wp.tile([8, 128 * E], f32)
            onesf = wp.tile([1, TB], f32)
            nc.gpsimd.memset(onesf[:], 1.0)
            nc.gpsimd.memset(SEf[:], 0.0)
            for e in range(E):
                nc.sync.dma_start(SEf[e:e + 1, 128 * e:128 * (e + 1)],
                                  onesf[:, 0:128])
            SE = wp.tile([8, 128 * E], f32r)
            nc.sync.dma_start(SE[:], SEf[:].bitcast(f32r))

            ones8 = wp.tile([8, 1], f32r)
            nc.sync.dma_start(ones8[:], onesf[:, 0:8].bitcast(f32r))
            ones1x8 = wp.tile([1, 8], f32r)
            nc.sync.dma_start(ones1x8[:], onesf[:, 0:8].bitcast(f32r))
            onesrow = wp.tile([1, TB], f32r)
            nc.sync.dma_start(onesrow[:], onesf[:].bitcast(f32r))

            # candidate-action scale row, repeated per expert: [128, E*A]
            scale_t = wp.tile([128, E * A], f32)
            for e in range(E):
                nc.gpsimd.memset(scale_t[:, A * e:A * (e + 1)],
                                 float(SCALING[e]))

            # prefetch block 0's X subtiles BEFORE the weight stream so the
            # transposes start immediately (gpsimd queue, ahead of weights)
            xn0 = []
            for s in range(4):
                t0_ = xnat_p.tile([128, H], f32r, tag="xn", name="xn")
                nc.gpsimd.dma_start(t0_[:],
                                    x_d[128 * s:128 * (s + 1), :].bitcast(f32r))
                xn0.append(t0_)

            # first-layer weights organized per feature-tile PAIR, DMA'd in
            # consumption order (router/value pair first) so the first L1
            # matmuls only wait on ~1MB, not the full 10.5MB.
            # wpair[p][:, 256k + 128j : 256k + 128(j+1)] = k-slice of ft 2p+j.
            # pair index: p = 0..7 experts, 8 = shared, 9 = router|value
            wpair = [wp.tile([128, 2048], f32r, tag=f"wpair{p}",
                             name=f"wpair{p}") for p in range(10)]

            def wslice(p, k, j):
                return wpair[p][:, 256 * k + 128 * j:256 * k + 128 * (j + 1)]

            # one big DMA per pair ([128, 8, 256] view of the [1024,256]
            # weight), issuance spread across otherwise-idle engine queues
            issuers = {0: nc.sync, 1: nc.scalar, 2: nc.sync,
                       3: nc.scalar, 4: nc.gpsimd, 5: nc.sync,
                       6: nc.scalar, 7: nc.gpsimd, 8: nc.gpsimd}
            rv_dst = wpair[9][:].rearrange("p (k c) -> p k c", k=8)
            nc.sync.dma_start(
                rv_dst[:, :, 0:128],
                rw1_d[:].rearrange("(p k) c -> p k c", k=8).bitcast(f32r))
            nc.scalar.dma_start(
                rv_dst[:, :, 128:256],
                vw1_d[:].rearrange("(p k) c -> p k c", k=8).bitcast(f32r))
            for p in range(9):
                dst = wpair[p][:].rearrange("p (k c) -> p k c", k=8)
                srcd = ew1_d[p] if p < 8 else sw1_d[:]
                issuers[p].dma_start(
                    dst[:],
                    srcd.rearrange("(p k) c -> p k c", k=8).bitcast(f32r))

            # expert second-layer weights [128, 16*32]
            w2 = wp.tile([128, 512], f32r)
            if BF16_G:
                w2b = wp.tile([128, 512], gdt, name="w2b")
            for e in range(E):
                for j in range(2):
                    nc.sync.dma_start(
                        w2[:, 32 * (2 * e + j):32 * (2 * e + j + 1)],
                        ew2_d[e, 128 * j:128 * (j + 1), :].bitcast(f32r))
            if BF16_G:
                nc.vector.tensor_copy(w2b[:], w2[:].bitcast(f32))
            sw2t = wp.tile([128, 64], f32r)
            if BF16_G:
                sw2b = wp.tile([128, 64], gdt, name="sw2b")
            for j in range(2):
                nc.sync.dma_start(sw2t[:, 32 * j:32 * (j + 1)],
                                  sw2_d[128 * j:128 * (j + 1), :].bitcast(f32r))
            if BF16_G:
                nc.vector.tensor_copy(sw2b[:], sw2t[:].bitcast(f32))
            w2u = w2b if BF16_G else w2
            sw2u = sw2b if BF16_G else sw2t
            rw2t = wp.tile([128, E], f32r)
            nc.sync.dma_start(rw2t[:], rw2_d[:, :].bitcast(f32r))
            vw2t = wp.tile([128, 1], f32r)
            nc.sync.dma_start(vw2t[:], vw2_d[:, :].bitcast(f32r))
            mw1t = wp.tile([32, 128], f32r)
            nc.sync.dma_start(mw1t[:], mw1_d[:, :].bitcast(f32r))
            mw2t = wp.tile([128, 32], f32r)
            nc.sync.dma_start(mw2t[:], mw2_d[:, :].bitcast(f32r))
            eb2t = wp.tile([8, 32], f32r)
            nc.sync.dma_start(eb2t[:], eb2_d[:, :].bitcast(f32r))
            sb2r = wp.tile([1, 32], f32r)
            nc.sync.dma_start(sb2r[:], sb2_d[None, :].bitcast(f32r))
            mb2r = wp.tile([1, 32], f32r)
            nc.sync.dma_start(mb2r[:], mb2_d[None, :].bitcast(f32r))

            # first-layer biases as [128,1] columns, added into psum by DVE
            # (cheaper than K=1 PE matmuls; ACT gelu then runs bias-free on
            # the whole pair). Column ft of b1c = bias for feature tile ft.
            b1c = wp.tile([128, FT], f32)
            for ftt in range(16):
                e, j = ftt // 2, ftt % 2
                nc.sync.dma_start(b1c[:, ftt:ftt + 1],
                                  eb1_d[e, 128 * j:128 * (j + 1)][:, None])
            for j in range(2):
                nc.sync.dma_start(b1c[:, 16 + j:17 + j],
                                  sb1_d[128 * j:128 * (j + 1)][:, None])
            nc.sync.dma_start(b1c[:, 18:19], rb1_d[:][:, None])
            rb2c = wp.tile([8, 1], f32)
            nc.sync.dma_start(rb2c[:], rb2_d[:][:, None])
            vb1c = wp.tile([128, 1], f32)
            nc.sync.dma_start(vb1c[:], vb1_d[:][:, None])
            vb2c = wp.tile([1, 1], f32)
            nc.sync.dma_start(vb2c[:], vb2_d[:][:, None])
            mb1c = wp.tile([128, 1], f32)
            nc.sync.dma_start(mb1c[:], mb1_d[:][:, None])
            nvb2 = wp.tile([1, 1], f32)
            nc.vector.tensor_scalar_mul(nvb2[:], vb2c[:], -1.0)

            # ---------------- per-block pipeline ----------------
            def l1pair(pair, xt, bias_a=True, bias_b=True):
                """first-layer matmuls for a feature-tile pair sharing one
                [128, 2*TB] psum (adjacent banks); per-feature biases added
                in-place by DVE so one bias-free activation op covers the
                pair."""
                psg = ps_g.tile([128, 2 * TB], f32, tag="psg", name="psg")
                for j, use_bias in ((0, bias_a), (1, bias_b)):
                    half = psg[:, TB * j:TB * (j + 1)]
                    ftt = 2 * pair + j
                    for k in range(8):
                        nc.tensor.matmul(half, wslice(pair, k, j), xt[k][:],
                                         start=(k == 0), stop=(k == 7))
                    if use_bias:
                        nc.vector.tensor_scalar_add(half, half,
                                                    b1c[:, ftt:ftt + 1])
                return psg

            for b in range(nblk):
                tok0 = b * TB

                # X in natural layout, then PE-transpose to XT [h, tok]
                if b == 0:
                    xn = xn0
                else:
                    xn = []
                    for s in range(4):
                        t = xnat_p.tile([128, H], f32r, tag="xn", name="xn")
                        nc.gpsimd.dma_start(
                            t[:], x_d[tok0 + 128 * s:tok0 + 128 * (s + 1), :]
                            .bitcast(f32r))
                        xn.append(t)
                xt = []
                for k in range(8):
                    pst = ps_t.tile([128, TB], f32r, tag="pst", name="pst")
                    for s in range(4):
                        xcols = xn[s][:].rearrange(
                            "t (p r) -> t r p", r=8)[:, k, :]
                        nc.tensor.matmul(
                            pst[:, 128 * s:128 * (s + 1)],
                            xcols, ident[:],
                            is_transpose=True,
                            start=(s == 0), stop=(s == 3))
                    t = xt_p.tile([128, TB], f32r, tag="xt", name="xt")
                    nc.vector.tensor_copy(t[:], pst[:])
                    xt.append(t)

                # router (ft 18, gelu+bias-mm) and value (ft 19, relu on DVE)
                psg_rv = l1pair(9, xt, bias_a=True, bias_b=False)
                g18 = g_p.tile([128, TB], f32r, tag="g18", bufs=1)
                nc.scalar.activation(g18[:], psg_rv[:, 0:TB], AF.Gelu)
                g19 = g_p.tile([128, TB], f32r, tag="g19", bufs=1)
                nc.vector.tensor_scalar(g19[:], psg_rv[:, TB:2 * TB],
                                        vb1c[:], 0.0, ALU.add, ALU.max)
                psr = ps_s.tile([8, TB], f32, tag="ps_small", name="psr")
                nc.tensor.matmul(psr[:], rw2t[:], g18[:], start=True, stop=True)
                expR = mp.tile([8, TB], f32r, tag="expR")
                nc.scalar.activation(expR[:], psr[:], AF.Exp, bias=rb2c[:])
                # value head: sigmoid(z) = 1/(1+exp(-z)) via the Exp table
                psv = ps_s.tile([1, TB], f32, tag="ps_small", name="psv")
                nc.tensor.matmul(psv[:], vw2t[:], g19[:], start=True, stop=True)
                ev = mp.tile([1, TB], f32, tag="ev", bufs=1)
                nc.scalar.activation(ev[:], psv[:], AF.Exp, bias=nvb2[:],
                                     scale=-1.0)
                dv = mp.tile([1, TB], f32, tag="dv", bufs=1)
                nc.vector.tensor_scalar_add(dv[:], ev[:], 1.0)
                vsig = mp.tile([1, TB], f32, tag="vsig", bufs=1)
                nc.vector.reciprocal_approx_fast(vsig[:], dv[:])
                stack = mp.tile([10, TB], f32r, tag="stack")
                nc.sync.dma_start(stack[8:9, :], vsig[:].bitcast(f32r))
                nc.sync.dma_start(stack[9:10, :],
                                  mask_d[tok0:tok0 + TB][None, :].bitcast(f32r))
                # 1/sum(exp) and normalized probs
                pss = ps_s.tile([1, TB], f32, tag="ps_small", name="pss")
                nc.tensor.matmul(pss[:], ones8[:], expR[:], start=True,
                                 stop=True)
                recf = mp.tile([1, TB], f32, tag="recf", bufs=1)
                nc.vector.reciprocal_approx_fast(recf[:], pss[:])
                recipS = mp.tile([1, TB], f32r, tag="recipS", bufs=1)
                nc.sync.dma_start(recipS[:], recf[:].bitcast(f32r))
                ps8 = ps_s.tile([8, TB], f32, tag="ps_small", name="ps8")
                nc.tensor.matmul(ps8[:], ones1x8[:], recipS[:], start=True,
                                 stop=True)
                nc.vector.tensor_mul(stack[0:8, :], expR[:], ps8[:])

                # probs/rtg/mask transposes early (stack rows 0-9 final here);
                # frees the block tail to just the fin path
                pos = []
                for s in range(4):
                    cols = slice(128 * s, 128 * (s + 1))
                    rows = slice(tok0 + 128 * s, tok0 + 128 * (s + 1))
                    pspo = ps_s.tile([128, 10], f32r, tag="ps_small",
                                     name="pspo")
                    nc.tensor.matmul(pspo[:], stack[:, cols],
                                     ident[0:10, 0:10], is_transpose=True,
                                     start=True, stop=True)
                    po = mp.tile([128, 10], f32, tag="po", bufs=5)
                    nc.vector.tensor_copy(po[:], pspo[:])
                    pos.append(po)
                    nc.sync.dma_start(probs_d[rows, :], po[:, 0:8])
                    nc.sync.dma_start(rtg_d[rows, :], po[:, 8:9])

                # experts: pre-scale hidden activations by probsA[e]
                # (broadcast via selector matmul), accumulate all expert
                # second-layer matmuls into one [32,TB] psum. The per-token
                # scalar commutes through the contraction, so this equals
                # sum_e probsA_e * (h1_e @ W2_e).
                pswe = ps_s.tile([32, TB], f32, tag="ps_small", name="pswe")
                for e in range(E):
                    psg2 = l1pair(e, xt)
                    g2 = g_p.tile([128, 2 * TB], gdt, tag="g2", name="g2", bufs=3)
                    nc.scalar.activation(g2[:], psg2[:], AF.Gelu)
                    pbps = ps_s.tile([128, TB], f32, tag="ps_small",
                                     name="pbps")
                    nc.tensor.matmul(pbps[:], SE[:, 128 * e:128 * (e + 1)],
                                     stack[0:8, :], start=True, stop=True)
                    gs = g_p.tile([128, 2 * TB], gdt, tag="gs", name="gs")
                    nc.vector.tensor_mul(
                        gs[:].rearrange("p (r n) -> p r n", r=2),
                        g2[:].rearrange("p (r n) -> p r n", r=2),
                        pbps[:].unsqueeze(1).broadcast_to([128, 2, TB]))
                    nc.tensor.matmul(pswe[:], w2u[:, 64 * e:64 * e + 32],
                                     gs[:, 0:TB], start=(e == 0), stop=False)
                    nc.tensor.matmul(pswe[:], w2u[:, 64 * e + 32:64 * e + 64],
                                     gs[:, TB:2 * TB], start=False, stop=False)
                nc.tensor.matmul(pswe[:], eb2t[:], stack[0:8, :], start=False,
                                 stop=True)
                wen = mp.tile([32, TB], f32r, tag="wen", bufs=1)
                nc.vector.tensor_copy(wen[:], pswe[:])

                # shared expert (bias via K=1 ones-matmul)
                psg2s = l1pair(8, xt)
                g2sh = g_p.tile([128, 2 * TB], gdt, tag="g2", name="g2sh", bufs=3)
                nc.scalar.activation(g2sh[:], psg2s[:], AF.Gelu)
                pssh = ps_s.tile([32, TB], f32, tag="ps_small", name="pssh")
                nc.tensor.matmul(pssh[:], sw2u[:, 0:32], g2sh[:, 0:TB],
                                 start=True, stop=False)
                nc.tensor.matmul(pssh[:], sw2u[:, 32:64], g2sh[:, TB:2 * TB],
                                 start=False, stop=False)
                nc.tensor.matmul(pssh[:], sb2r[:], onesrow[:], start=False,
                                 stop=True)

                # moe = shared + weighted_expert; residual MLP
                moe = mp.tile([32, TB], f32r, tag="moe", bufs=1)
                nc.vector.tensor_add(moe[:], pssh[:], wen[:])
                psr1 = ps_s.tile([128, TB], f32, tag="ps_small", name="psr1")
                nc.tensor.matmul(psr1[:], mw1t[:], moe[:], start=True,
                                 stop=True)
                r1 = g_p.tile([128, TB], f32r, tag="r1")
                nc.scalar.activation(r1[:], psr1[:], AF.Gelu, bias=mb1c[:])
                psr2 = ps_s.tile([32, TB], f32, tag="ps_small", name="psr2")
                nc.tensor.matmul(psr2[:], mw2t[:], r1[:], start=True,
                                 stop=False)
                nc.tensor.matmul(psr2[:], mb2r[:], onesrow[:], start=False,
                                 stop=True)
                fin = mp.tile([32, TB], f32r, tag="fin", bufs=1)
                nc.vector.tensor_add(fin[:], psr2[:], wen[:])

                # transpose outputs back to token-major, apply mask, store
                for s in range(4):
                    cols = slice(128 * s, 128 * (s + 1))
                    rows = slice(tok0 + 128 * s, tok0 + 128 * (s + 1))
                    psf = ps_s.tile([128, 32], f32r, tag="ps_small",
                                    name="psf")
                    nc.tensor.matmul(psf[:], fin[:, cols], id32[:],
                                     is_transpose=True, start=True, stop=True)
                    fo = mp.tile([128, 32], f32, tag="fo")
                    nc.vector.tensor_scalar_mul(fo[:], psf[:], pos[s][:, 9:10])
                    nc.sync.dma_start(fin_d[rows, :], fo[:])

                # candidate actions (independent path): one broadcast DVE mul
                for s in range(4):
                    rows = slice(tok0 + 128 * s, tok0 + 128 * (s + 1))
                    bt = mp.tile([128, A], f32, tag="bt")
                    nc.sync.dma_start(bt[:], ba_d[rows, :])
                    cs = mp.tile([128, E * A], f32, tag="cs", bufs=1)
                    nc.vector.tensor_mul(
                        cs[:].rearrange("p (e a) -> p e a", e=E),
                        bt[:].unsqueeze(1).broadcast_to([128, E, A]),
                        scale_t[:].rearrange("p (e a) -> p e a", e=E))
                    nc.sync.dma_start(cand_d[rows, :], cs[:])

    nc.compile()
    return nc


def _get_compiled(ntok=NTOK):
    if ntok not in _compiled:
        _compiled[ntok] = build_nc(ntok)
    return _compiled[ntok]


def _run(inputs, trace=False, tmpdir=None):
    from concourse.bass_utils import run_bass_kernel_spmd

    nc = _get_compiled()

    state_rep = np.ascontiguousarray(inputs["state_rep"], dtype=np.float32)
    base_action = np.ascontiguousarray(inputs["base_action"], dtype=np.float32)
    attention_mask = np.ascontiguousarray(inputs["attention_mask"],
                                          dtype=np.float32)
    wmap = {k: np.ascontiguousarray(inputs[k], dtype=np.float32)
            for k in ("ew1", "eb1", "ew2", "eb2", "sw1", "sb1", "sw2", "sb2",
                      "rw1", "rb1", "rw2", "rb2", "vw1", "vb1", "vw2", "vb2",
                      "mw1", "mb1", "mw2", "mb2")}
    wmap["vw2"] = wmap["vw2"].reshape(RH, 1)
    wmap["vb2"] = wmap["vb2"].reshape(1)

    bpc = B // NCORES  # batches per core
    in_maps = []
    for c in range(NCORES):
        bs = slice(bpc * c, bpc * (c + 1))
        m = dict(wmap)
        m["x"] = state_rep[bs].reshape(NTOK, H)
        m["ba"] = base_action[bs].reshape(NTOK, A)
        m["mask"] = attention_mask[bs].reshape(NTOK)
        in_maps.append(m)

    res = run_bass_kernel_spmd(nc, in_maps, list(range(NCORES)),
                               trace=trace, tmpdir=tmpdir)

    fin = np.concatenate([res.results[c]["fin"] for c in range(NCORES)])
    cand = np.concatenate([res.results[c]["cand"] for c in range(NCORES)])
    probs = np.concatenate([res.results[c]["probs"] for c in range(NCORES)])
    rtg = np.concatenate([res.results[c]["rtg"] for c in range(NCORES)])

    out = (fin.reshape(B, S, A), cand.reshape(B, S, E, A),
           probs.reshape(B, S, E), rtg.reshape(B, S, 1))
    return out, res


def kernel(state_rep, base_action, attention_mask,
           sw1, sb1, sw2, sb2, ew1, eb1, ew2, eb2,
           rw1, rb1, rw2, rb2, mw1, mb1, mw2, mb2,
           vw1, vb1, vw2, vb2):
    out, _ = _run(dict(
        state_rep=state_rep, base_action=base_action,
        attention_mask=attention_mask,
        sw1=sw1, sb1=sb1, sw2=sw2, sb2=sb2, ew1=ew1, eb1=eb1, ew2=ew2,
        eb2=eb2, rw1=rw1, rb1=rb1, rw2=rw2, rb2=rb2, mw1=mw1, mb1=mb1,
        mw2=mw2, mb2=mb2, vw1=vw1, vb1=vb1, vw2=vw2, vb2=vb2))
    return out
